# revision 23
# baseline (speedup 1.0000x reference)
"""AttentionBlock (GroupNorm -> 1x1 qkv -> 4-head attention over 64x64 -> proj -> residual)
distributed over 8 Trainium2 NeuronCores.

Sharding: 8 shards = batch(4) x query-half(2). Each core receives the full
[256, 4096] feature map of its batch element (columns rolled so its own query
half is always columns 0:2048 -> identical SPMD graph on every core).

Key optimizations over the bf16 baseline (366 us):
- GroupNorm folded into runtime-scaled qkv weights (W' = a_c * W, bias via tiny
  PE matvecs); x cast to bf16 once, no normalized-x materialization.
- The whole attention inner loop runs in fp8e4 DoubleRow matmuls (2x contraction
  per column): scores contract q against both k head-planes with a zero-plane
  masking trick (q8 planes [q0,q1,0,q2,q3]; head h reads plane pair (h-plane,
  zero-plane) so the unwanted head contributes nothing), and attn@v contracts
  two 128-key j-chunks per matmul. K>=65 tiles keep the PE in full-rate 128-row
  mode (K<=64 streams at half rate on TRN2).
- exp(scores) split between Scalar (exact Exp -> fp8, table-free) and Vector
  (Schraudolph: scores arrive pre-scaled by 11.5416 = 8/ln2 via the host q
  scale, so max(psum + c0, 0) cast to uint8 IS the fp8 bit pattern of
  e^(s - DELTA)). GPSIMD cannot read PSUM, so it handles SBUF-only work.
- softmax denominator via a ones column in the fp8 v^T (row 64 of the DoubleRow
  pv matmul), normalization off the critical path via a DRAM-broadcast round
  trip, v-bias and all GroupNorm bias terms pushed into the proj bias.
"""

import sys

sys.path.insert(0, "/opt/trn_rl_repo")

import numpy as np
import ml_dtypes

import concourse.bass as bass
import concourse.tile as tile
from concourse import bacc, mybir

# Problem geometry (hardcoded per harness contract)
B, C, H, W = 4, 256, 64, 64
N = H * W              # 4096 spatial positions
HEADS = 4
D = C // HEADS         # 64
GROUPS = 8
EPS = 1e-5
NCORES = 8
NI = N // 2            # 2048 queries per core
IB = 1024              # i-block
JC = 128               # j-chunk (keys per scores matmul)
NJ = N // JC           # 32 j-chunks
NJP = NJ // 2          # 16 j-chunk pairs (DoubleRow contracts a pair)

DELTA = 2.0                       # score shift: exp(s - DELTA), cancels in softmax
SCHR_A = 8.0 / float(np.log(2))   # 11.5416 = fp8e4 bits per e-fold
SCHR_C0 = 56.3                    # bits = A*(s - DELTA) + C0

F32 = mybir.dt.float32
BF16 = mybir.dt.bfloat16
F8 = mybir.dt.float8e4
U8 = mybir.dt.uint8
DR = mybir.MatmulPerfMode.DoubleRow

# q8 planes: [q0, q1, zero, q2, q3]; head h -> (first plane, plane stride)
Q_PLANES = [(0, 2), (1, 1), (2, 1), (2, 2)]

_CACHE = {}


def _build_nc():
    nc = bacc.Bacc("TRN2", target_bir_lowering=False, debug=False,
                   num_devices=NCORES)

    x_ext = nc.declare_dram_parameter("x", [C, N], BF16, isOutput=False)
    # columns: [ (A/8)*qT (256) | kT (256) | 16*vT (256) ],  A = 8/ln2
    wqkvT_ext = nc.declare_dram_parameter("wqkvT", [C, 3 * C], BF16, isOutput=False)
    wpT_ext = nc.declare_dram_parameter("wpT", [C, C], BF16, isOutput=False)
    qkb_ext = nc.declare_dram_parameter("qkb", [128, 4], F32, isOutput=False)
    pb_ext = nc.declare_dram_parameter("pb", [128, 2], F32, isOutput=False)
    gnw_ext = nc.declare_dram_parameter("gnw", [128, 2], F32, isOutput=False)
    gnb_ext = nc.declare_dram_parameter("gnb", [128, 2], F32, isOutput=False)
    oneh_ext = nc.declare_dram_parameter("oneh", [C, GROUPS], F32, isOutput=False)
    onehT_ext = nc.declare_dram_parameter("onehT", [GROUPS, C], F32, isOutput=False)
    zz_ext = nc.declare_dram_parameter("zz", [128, N], F8, isOutput=False)
    out_ext = nc.declare_dram_parameter("out", [C, NI], F32, isOutput=True)

    with tile.TileContext(nc) as tc:
        with (
            tc.tile_pool(name="persist", bufs=1) as per,
            tc.tile_pool(name="etp", bufs=8) as etp,
            tc.tile_pool(name="ep", bufs=2) as ep,
            tc.tile_pool(name="yp", bufs=2) as yp,
            tc.tile_pool(name="dp", bufs=2, space="DRAM") as dp,
            tc.tile_pool(name="ps", bufs=3, space="PSUM") as ps,
            tc.tile_pool(name="pv", bufs=2, space="PSUM") as pvp,
        ):
            # ---- persistent SBUF tensors ----
            x_sb = [per.tile([128, N], BF16, tag=f"x{t}", name=f"x{t}") for t in range(2)]
            q8 = per.tile([128, 5, NI], F8, tag="q8")
            k8 = per.tile([128, 2, N], F8, tag="k8")
            # v^T per j-chunk: [j%128, jchunk, head, 0:64 v | 64 ones | 65 zero]
            vton = per.tile([128, NJ, HEADS, 128], F8, tag="vton")
            att_sb = [per.tile([128, NI], BF16, tag=f"att{t}", name=f"att{t}") for t in range(2)]
            wraw_sb = [per.tile([128, 3 * C], BF16, tag=f"wr{t}", name=f"wr{t}") for t in range(2)]
            wsc_sb = [per.tile([128, 3 * C], BF16, tag=f"ws{t}", name=f"ws{t}") for t in range(2)]
            wpT_sb = [per.tile([128, C], BF16, tag=f"wp{t}", name=f"wp{t}") for t in range(2)]
            qkb_sb = per.tile([128, 4], F32, tag="qkb")
            biasqk_sb = per.tile([128, 4], F32, tag="biasqk")
            pb_sb = per.tile([128, 2], F32, tag="pb")
            pbrt_sb = per.tile([128, 2], F32, tag="pbrt")
            gnw_sb = per.tile([128, 2], F32, tag="gnw")
            gnb_sb = per.tile([128, 2], F32, tag="gnb")
            oneh_sb = [per.tile([128, GROUPS], F32, tag=f"oneh{t}", name=f"oneh{t}") for t in range(2)]
            onehT_sb = per.tile([GROUPS, C], F32, tag="onehT")
            eps_sb = per.tile([GROUPS, 1], F32, tag="eps")
            ndel_sb = per.tile([128, 1], F32, tag="ndel")
            ab_sb = [per.tile([128, 2], F32, tag=f"ab{t}", name=f"ab{t}") for t in range(2)]
            bvec_sb = [per.tile([128, 1], BF16, tag=f"bv{t}", name=f"bv{t}") for t in range(2)]
            vx16_sb = [per.tile([128, 1], BF16, tag=f"vx{t}", name=f"vx{t}") for t in range(2)]
            gst_sb = per.tile([GROUPS, 4], F32, tag="gst")

            ones1 = per.tile([1, D], F32, tag="ones1")
            nc.vector.memset(ones1[:], 1.0)
            nc.vector.memset(eps_sb[:], EPS)
            nc.vector.memset(ndel_sb[:], -DELTA)
            nc.vector.memset(vton[:, :, :, D : D + 1], 1.0)
            nc.vector.memset(vton[:, :, :, D + 1 : D + 2], 0.0)

            # ---- x DMA + GroupNorm statistics + bf16 cast ----
            stats = [per.tile([128, 8, 6], F32, tag=f"st{t}", name=f"st{t}") for t in range(2)]
            mv = [per.tile([128, 4], F32, tag=f"mv{t}", name=f"mv{t}") for t in range(2)]
            x_queues = {(0, 0): nc.sync, (0, 1): nc.sync,
                        (0, 2): nc.sync, (0, 3): nc.scalar,
                        (1, 0): nc.gpsimd, (1, 1): nc.gpsimd,
                        (1, 2): nc.gpsimd, (1, 3): nc.scalar}
            for t in range(2):
                cs = slice(t * 128, (t + 1) * 128)
                for ch in range(4):
                    chs = slice(ch * IB, (ch + 1) * IB)
                    x_queues[(t, ch)].dma_start(out=x_sb[t][:, chs], in_=x_ext[cs, chs])
                    for s in range(2):
                        sub = ch * 2 + s
                        nc.vector.bn_stats(
                            out=stats[t][:, sub, :],
                            in_=x_sb[t][:, sub * 512 : (sub + 1) * 512],
                        )
                if t == 1:
                    # weights / zeros / small inputs ride behind x
                    for t2 in range(2):
                        cs2 = slice(t2 * 128, (t2 + 1) * 128)
                        nc.sync.dma_start(out=wraw_sb[t2][:], in_=wqkvT_ext[cs2, :])
                        nc.scalar.dma_start(out=wpT_sb[t2][:], in_=wpT_ext[cs2, :])
                        nc.scalar.dma_start(out=oneh_sb[t2][:], in_=oneh_ext[cs2, :])
                    nc.scalar.dma_start(out=qkb_sb[:], in_=qkb_ext[:])
                    nc.scalar.dma_start(out=pb_sb[:], in_=pb_ext[:])
                    nc.sync.dma_start(out=gnw_sb[:], in_=gnw_ext[:])
                    nc.sync.dma_start(out=gnb_sb[:], in_=gnb_ext[:])
                    nc.scalar.dma_start(out=onehT_sb[:], in_=onehT_ext[:])
                    nc.gpsimd.dma_start(out=k8[:, 1, :], in_=zz_ext[:])
                    nc.gpsimd.dma_start(out=q8[:, 2, :], in_=zz_ext[:, 0:NI])
                    nc.gpsimd.dma_start(out=q8[64:128, 0, :], in_=zz_ext[64:128, 0:NI])
                    nc.gpsimd.dma_start(out=q8[0:64, 1, :], in_=zz_ext[0:64, 0:NI])
                    nc.gpsimd.dma_start(out=q8[64:128, 3, :], in_=zz_ext[64:128, 0:NI])
                    nc.gpsimd.dma_start(out=q8[0:64, 4, :], in_=zz_ext[0:64, 0:NI])
                nc.vector.bn_aggr(out=mv[t][:, 0:2], in_=stats[t][:])
                nc.vector.tensor_copy(mv[t][:, 2:3], mv[t][:, 0:1])
                nc.vector.tensor_mul(mv[t][:, 3:4], mv[t][:, 0:1], mv[t][:, 0:1])
                nc.vector.tensor_add(mv[t][:, 3:4], mv[t][:, 1:2], mv[t][:, 3:4])

            # group means of (mean, E[x^2]): [8, 2]
            gpt = ps.tile([128, 2, 512], F32, tag="ps", name="gn_ps")
            gp = gpt[0:GROUPS, 0, 0:2]
            for t in range(2):
                nc.tensor.matmul(
                    gp, oneh_sb[t][:], mv[t][:, 2:4],
                    start=(t == 0), stop=(t == 1),
                )
            # gst columns: 0=mean_g, 1=rstd_g; scratch 2=var, 3=std
            nc.vector.tensor_copy(gst_sb[:, 0:1], gp[:, 0:1])
            nc.vector.tensor_mul(gst_sb[:, 2:3], gst_sb[:, 0:1], gst_sb[:, 0:1])
            nc.vector.tensor_sub(gst_sb[:, 2:3], gp[:, 1:2], gst_sb[:, 2:3])
            # rstd = exp(-0.5*ln(var+eps)); Ln/Exp share one ACT table set
            nc.scalar.activation(
                out=gst_sb[:, 3:4], in_=gst_sb[:, 2:3],
                func=mybir.ActivationFunctionType.Ln,
                bias=eps_sb[:], scale=1.0,
            )
            nc.vector.tensor_scalar_mul(
                out=gst_sb[:, 3:4], in0=gst_sb[:, 3:4], scalar1=-0.5
            )
            nc.scalar.activation(
                out=gst_sb[:, 1:2], in_=gst_sb[:, 3:4],
                func=mybir.ActivationFunctionType.Exp, scale=1.0,
            )

            # broadcast (mean_g, rstd_g) to channels; a = rstd*gn_w,
            # b = gn_b - mean*a; scale weights: ws = a_c * wraw (on GPSIMD)
            bct = ps.tile([128, 2, 512], F32, tag="ps", name="gn_bc")
            for t in range(2):
                bc = bct[:, t, 0:2]
                nc.tensor.matmul(
                    bc, onehT_sb[:, t * 128 : (t + 1) * 128], gst_sb[:, 0:2],
                    start=True, stop=True,
                )
                nc.vector.tensor_mul(ab_sb[t][:, 0:1], bc[:, 1:2], gnw_sb[:, t : t + 1])
                nc.vector.tensor_mul(ab_sb[t][:, 1:2], bc[:, 0:1], ab_sb[t][:, 0:1])
                nc.vector.tensor_sub(ab_sb[t][:, 1:2], gnb_sb[:, t : t + 1], ab_sb[t][:, 1:2])
                nc.vector.tensor_copy(bvec_sb[t][:], ab_sb[t][:, 1:2])
                if t == 0:
                    nc.scalar.activation(
                        out=wsc_sb[t][:], in_=wraw_sb[t][:],
                        func=mybir.ActivationFunctionType.Copy,
                        scale=ab_sb[t][:, 0:1])
                else:
                    nc.vector.tensor_scalar_mul(
                        out=wsc_sb[t][:], in0=wraw_sb[t][:],
                        scalar1=ab_sb[t][:, 0:1])

            # runtime bias matvecs: bp[:, 0:6] = wraw.T @ b_vec (per o-tile)
            bpt = ps.tile([128, 2, 512], F32, tag="ps", name="bias_mv")
            bp = bpt[:, 0, 0:6]
            for ot in range(6):
                for t in range(2):
                    nc.tensor.matmul(
                        bp[:, ot : ot + 1],
                        wraw_sb[t][:, ot * 128 : (ot + 1) * 128],
                        bvec_sb[t][:],
                        start=(t == 0), stop=(t == 1),
                    )
            nc.vector.tensor_add(biasqk_sb[:], qkb_sb[:], bp[:, 0:4])
            for t in range(2):
                nc.vector.tensor_copy(vx16_sb[t][:], bp[:, 4 + t : 5 + t])
            # pb_rt = pb + (proj_w/16) @ (16 Wv b_vec)
            pp2t = ps.tile([128, 2, 512], F32, tag="ps", name="pb_mv")
            pp2 = pp2t[:, 0, 0:2]
            for ot in range(2):
                for t in range(2):
                    nc.tensor.matmul(
                        pp2[:, ot : ot + 1],
                        wpT_sb[t][:, ot * 128 : (ot + 1) * 128],
                        vx16_sb[t][:],
                        start=(t == 0), stop=(t == 1),
                    )
            nc.vector.tensor_add(pbrt_sb[:], pb_sb[:], pp2[:, 0:2])

            # ---- emission helpers ----
            def q_rhs(h, isl):
                p0, st = Q_PLANES[h]
                base = q8[:, 0, isl]
                return bass.AP(
                    tensor=base.tensor,
                    offset=base.offset + p0 * NI,
                    ap=[list(base.ap[0])] + [[st * NI, 2]] +
                       [list(a) for a in base.ap[1:]],
                )

            def qkv_tiles(ot, blocks=None):
                # ot 0,1 = q o-tiles (head pairs); 2,3 = k o-tiles
                ncols = NI if ot < 2 else N
                wcols = slice(ot * 128, (ot + 1) * 128)
                for nb in (range(ncols // IB) if blocks is None else blocks):
                    pp = ps.tile([128, 2, 512], F32, tag="ps", name=f"qkv{ot}_{nb}")
                    for cc in range(2):
                        for nh in range(2):
                            nsl = slice(nb * IB + nh * 512, nb * IB + (nh + 1) * 512)
                            nc.tensor.matmul(
                                pp[:, nh, :], wsc_sb[cc][:, wcols], x_sb[cc][:, nsl],
                                start=(cc == 0), stop=(cc == 1),
                            )
                    nbsl = slice(nb * IB, (nb + 1) * IB)
                    ppf = pp[:].rearrange("p a b -> p (a b)")
                    if ot >= 2:
                        nc.scalar.activation(
                            out=k8[:, ot - 2, nbsl], in_=ppf,
                            func=mybir.ActivationFunctionType.Identity,
                            scale=1.0, bias=biasqk_sb[:, ot : ot + 1],
                        )
                    else:
                        # heads 2*ot (rows 0:64) and 2*ot+1 (rows 64:128) go to
                        # their own q8 planes
                        pl0 = [0, 3][ot]
                        pl1 = [1, 4][ot]
                        nc.vector.tensor_scalar_add(
                            out=q8[0:64, pl0, nbsl], in0=ppf[0:64],
                            scalar1=biasqk_sb[0:64, ot : ot + 1],
                        )
                        nc.vector.tensor_scalar_add(
                            out=q8[64:128, pl1, nbsl], in0=ppf[64:128],
                            scalar1=biasqk_sb[64:128, ot : ot + 1],
                        )

            def vt_pair(jp2):
                # v^T for j-chunks (2*jp2, 2*jp2+1): two [j128, 256] matmul
                # groups -> one fp8 copy
                pj = ps.tile([128, 2, 512], F32, tag="ps", name=f"vt{jp2}")
                for jc in range(2):
                    jsl = slice((2 * jp2 + jc) * JC, (2 * jp2 + jc + 1) * JC)
                    for cc in range(2):
                        nc.tensor.matmul(
                            pj[:, jc, 0:256], x_sb[cc][:, jsl],
                            wsc_sb[cc][:, 512:768],
                            start=(cc == 0), stop=(cc == 1),
                        )
                nc.scalar.activation(
                    out=vton[:, 2 * jp2 : 2 * jp2 + 2, :, 0:D],
                    in_=pj[:, :, 0:256].rearrange("p a (h d) -> p a h d", h=HEADS),
                    func=mybir.ActivationFunctionType.Copy, scale=1.0,
                )

            # exp rotation: ACT exact exp -> fp8; DVE Schraudolph (psum is
            # A*s'; the fp8 bits of e^(s'-DELTA) are max(psum+c0, 0) cast to
            # uint8 -- saturating on hw, max() keeps the sim's wrapping cast
            # safe too). 18:14 ratio balances measured engine loads.
            exp_rota = ([True, False] * 14) + [True] * 4
            exp_rota_early = [True, False]
            exp_ctr = [0]
            exp_early = [True]

            def emit_exp(et, scp):
                rota = exp_rota_early if exp_early[0] else exp_rota
                on_act = rota[exp_ctr[0] % len(rota)]
                exp_ctr[0] += 1
                if on_act:
                    nc.scalar.activation(
                        out=et[:], in_=scp[:],
                        func=mybir.ActivationFunctionType.Exp,
                        scale=1.0 / SCHR_A, bias=ndel_sb[:],
                    )
                else:
                    nc.vector.tensor_scalar(
                        out=et[:].bitcast(U8), in0=scp[:],
                        scalar1=SCHR_C0 - SCHR_A * DELTA, scalar2=0.0,
                        op0=mybir.AluOpType.add, op1=mybir.AluOpType.max,
                    )

            def attn_head(ib, h, with_vt, mid_cb=None, post_ic=None,
                          defer_last=False):
                ht = h // 2
                prow = slice((h % 2) * D, (h % 2) * D + D)
                deferred = []
                for ic in range(2):
                    isl = slice(ib * IB + ic * 512, ib * IB + (ic + 1) * 512)
                    pv = pvp.tile([D + 2, 512], F32, tag="pv",
                                  name=f"pv{ib}_{h}_{ic}")
                    for jp in range(NJP):
                        scp = ps.tile([128, 2, 512], F32, tag="ps",
                                      name=f"sc{ib}_{h}_{jp}_{ic}")
                        et = etp.tile([128, 2, 512], F8, tag="et",
                                      name=f"et{ib}_{h}_{jp}_{ic}")
                        for jc in range(2):
                            j = 2 * jp + jc
                            jsl = slice(j * JC, (j + 1) * JC)
                            nc.tensor.matmul(
                                scp[:, jc, :], k8[:, :, jsl], q_rhs(h, isl),
                                start=True, stop=True, perf_mode=DR,
                            )
                        emit_exp(et, scp)
                        if with_vt and ic == 0 and 3 <= jp <= 14:
                            vt_pair(jp + 1)
                        if mid_cb is not None:
                            mid_cb(jp, ic)
                        nc.tensor.matmul(
                            pv[:], vton[:, 2 * jp : 2 * jp + 2, h, 0 : D + 2],
                            et[:],
                            start=(jp == 0), stop=(jp == NJP - 1),
                            perf_mode=DR,
                        )
                    def epilogue(ic=ic, pv=pv):
                        _epilogue(ib, h, ht, prow, ic, pv, post_ic)
                    if defer_last and ic == 1:
                        deferred.append(epilogue)
                    else:
                        epilogue()
                return deferred

            def _epilogue(ib, h, ht, prow, ic, pv, post_ic):
                    isl = slice(ib * IB + ic * 512, ib * IB + (ic + 1) * 512)
                    # per-half epilogue: normalize off the critical path.
                    # h<3: den broadcast across 64 partitions via a DRAM round
                    # trip. h==3 (proj waits on it): recip the den row in SBUF
                    # and broadcast through the PE with a ones column instead.
                    if h == 3:
                        denr = ep.tile([1, 512], F32, tag="denr",
                                       name=f"denr{ib}_{h}_{ic}")
                        nc.scalar.activation(
                            out=denr[:], in_=pv[D : D + 1, :],
                            func=mybir.ActivationFunctionType.Copy, scale=1.0)
                        nc.vector.reciprocal_approx_fast(out=denr[:], in_=denr[:])
                        rbp = ps.tile([128, 2, 512], F32, tag="ps",
                                      name=f"rbp{ib}_{h}_{ic}")
                        nc.tensor.matmul(rbp[0:D, 0, :], ones1[:], denr[:],
                                         start=True, stop=True)
                        pvs3 = ep.tile([D, 512], F32, tag="pvs3",
                                       name=f"pvs3{ib}_{h}_{ic}")
                        nc.vector.tensor_copy(pvs3[:], pv[0:D, :])
                        nc.vector.tensor_mul(att_sb[ht][prow, isl],
                                             pvs3[:], rbp[0:D, 0, :])
                    else:
                        pvs = ep.tile([D + 2, 512], F32, tag="pvs",
                                      name=f"pvs{ib}_{h}_{ic}")
                        if ic == 0:
                            nc.scalar.activation(
                                out=pvs[:], in_=pv[:],
                                func=mybir.ActivationFunctionType.Copy, scale=1.0)
                        else:
                            nc.vector.tensor_copy(pvs[:], pv[:])
                        dent = dp.tile([1, 512], F32, tag="dent",
                                       name=f"den{ib}_{h}_{ic}")
                        nc.sync.dma_start(out=dent[:], in_=pvs[D : D + 1, :])
                        rbs = ep.tile([D, 512], F32, tag="rbs",
                                      name=f"rbs{ib}_{h}_{ic}")
                        dbc = bass.AP(
                            tensor=dent.tensor, offset=dent.offset,
                            ap=[[0, D]] + [list(a) for a in dent.ap[1:]],
                        )
                        nc.sync.dma_start(out=rbs[:], in_=dbc)
                        nc.vector.reciprocal_approx_fast(out=rbs[:], in_=rbs[:])
                        nc.gpsimd.tensor_mul(att_sb[ht][prow, isl],
                                             pvs[0:D, :], rbs[:])
                    if post_ic is not None:
                        post_ic(ic)

            def proj_part(ib, cc, ypart_tiles, halves=(0, 1), final=False):
                for ot in range(2):
                    pp = ps.tile([128, 2, 512], F32, tag="ps",
                                 name=f"pj{ib}_{cc}_{ot}_{halves[0]}")
                    wcols = slice(ot * 128, (ot + 1) * 128)
                    for nh in halves:
                        asl = slice(ib * IB + nh * 512, ib * IB + (nh + 1) * 512)
                        nc.tensor.matmul(
                            pp[:, nh, :], wpT_sb[cc][:, wcols], att_sb[cc][:, asl],
                            start=True, stop=True,
                        )
                    for nh in halves:
                        psl = pp[:, nh, :]
                        ysl = slice(ib * IB + nh * 512, ib * IB + (nh + 1) * 512)
                        csl = slice(nh * 512, (nh + 1) * 512)
                        if cc == 0:
                            if nh == halves[0]:
                                yt = yp.tile([128, IB], F32, tag=f"ypart{ot}",
                                             name=f"ypart{ib}_{ot}")
                                ypart_tiles.append(yt)
                            yt = ypart_tiles[ot]
                            nc.scalar.activation(
                                out=yt[:, csl], in_=psl,
                                func=mybir.ActivationFunctionType.Identity,
                                scale=1.0, bias=pbrt_sb[:, ot : ot + 1])
                        else:
                            y_sb = yp.tile([128, 512], F32, tag="y",
                                           name=f"y{ib}_{ot}_{nh}")
                            add1 = nc.vector if final else nc.gpsimd
                            add1.tensor_add(y_sb[:], ypart_tiles[ot][:, csl],
                                            x_sb[ot][:, ysl])
                            nc.vector.tensor_add(y_sb[:], y_sb[:], psl)
                            nc.sync.dma_start(
                                out=out_ext[ot * 128 : (ot + 1) * 128, ysl],
                                in_=y_sb[:])

            # ---- schedule ----
            qkv_tiles(0)        # q heads 0,1 (q8 planes 0,1)
            for jp2 in range(4):
                vt_pair(jp2)    # v chunks for jp 0..3
            qkv_tiles(2, [0])   # k heads 0,1, first block
            yparts = {}
            pending_ep = []
            for ib in range(NI // IB):
                yparts[ib] = []
                for h in range(HEADS):
                    if ib == 0 and h == 0:
                        # k plane0 blocks 1-3 stream in ahead of first use
                        mid = (lambda jp, ic: qkv_tiles(2, [1 + jp // 4])
                               if (ic == 0 and jp in (0, 4, 8)) else None)
                    elif ib == 0 and h == 1:
                        # q planes 3,4 and k plane1 before heads 2,3
                        def mid(jp, ic):
                            if ic == 0 and jp in (0, 4, 8):
                                qkv_tiles(3, [1 + jp // 4] if jp else [0, 1])
                            elif ic == 1 and jp in (0, 8):
                                qkv_tiles(1, [jp // 8])
                    elif ib > 0 and h == 0:
                        mid = (lambda jp, ic, p=ib - 1: proj_part(p, 1, yparts[p])
                               if (jp == 4 and ic == 0) else None)
                    elif h == 3:
                        mid = (lambda jp, ic, p=ib: proj_part(p, 0, yparts[p])
                               if (jp == 4 and ic == 0) else None)
                    else:
                        mid = None
                    last = (ib == NI // IB - 1 and h == 3)
                    if last:
                        prev_mid = mid
                        def mid(jp, ic, pm=prev_mid, p=ib):
                            if pm is not None:
                                pm(jp, ic)
                            if ic == 1 and jp == 8:
                                proj_part(p, 1, yparts[p], halves=(0,), final=True)
                        post = (lambda ic, p=ib: proj_part(
                            p, 1, yparts[p], halves=(1,), final=True)
                            if ic == 1 else None)
                    else:
                        post = None
                    prev2 = mid
                    def mid(jp, ic, pm=prev2, eps_=tuple(pending_ep)):
                        if ic == 0 and jp == 2:
                            for e in eps_:
                                e()
                        if pm is not None:
                            pm(jp, ic)
                    pending_ep = attn_head(
                        ib, h, with_vt=(ib == 0 and h == 0), mid_cb=mid,
                        post_ic=post, defer_last=not last)
                    if ib == 0 and h == 1:
                        exp_early[0] = False

    nc.compile()
    return nc


def _prep_in_maps(x, gn_w, gn_b, qkv_w, qkv_b, proj_w, proj_b):
    x = np.ascontiguousarray(np.asarray(x, np.float32)).reshape(B, C, N)
    qkv_w = np.asarray(qkv_w, np.float32)
    qkv_b = np.asarray(qkv_b, np.float32)
    proj_w = np.asarray(proj_w, np.float32)
    proj_b = np.asarray(proj_b, np.float32)
    gn_w = np.asarray(gn_w, np.float32)
    gn_b = np.asarray(gn_b, np.float32)

    bf = ml_dtypes.bfloat16
    qs = SCHR_A / 8.0             # fold D^-0.5 and the Schraudolph slope into q
    wq = qkv_w[:C] * qs
    wk = qkv_w[C : 2 * C]
    wv = 16.0 * qkv_w[2 * C :]    # scale v for fp8; /16 folded into wpT
    wqkvT = np.ascontiguousarray(np.concatenate([wq.T, wk.T, wv.T], axis=1)).astype(bf)
    wpT = np.ascontiguousarray(proj_w.T / 16.0).astype(bf)
    qkb = np.ascontiguousarray(
        np.concatenate([(qkv_b[:C] * qs).reshape(2, 128).T,
                        qkv_b[C : 2 * C].reshape(2, 128).T], axis=1))
    # fold v-bias through proj: proj(att + vb) = proj(att) + proj_w @ vb
    pb_eff = proj_b + proj_w.astype(np.float64) @ qkv_b[2 * C :].astype(np.float64)
    pb = np.ascontiguousarray(pb_eff.astype(np.float32).reshape(2, 128).T)
    gnw2 = np.ascontiguousarray(gn_w.reshape(2, 128).T)
    gnb2 = np.ascontiguousarray(gn_b.reshape(2, 128).T)
    cidx = np.arange(C)
    oneh = (cidx[:, None] // 32 == np.arange(GROUPS)[None, :]).astype(np.float32) / 32.0
    onehT = np.ascontiguousarray(oneh.T * 32.0)
    zz = np.zeros((128, N), ml_dtypes.float8_e4m3)

    shared = {
        "wqkvT": wqkvT, "wpT": wpT, "qkb": qkb, "pb": pb,
        "gnw": gnw2, "gnb": gnb2, "oneh": oneh, "onehT": onehT, "zz": zz,
    }
    in_maps = []
    for core in range(NCORES):
        bi, half = divmod(core, 2)
        xb = x[bi]
        if half:
            xs = np.concatenate([xb[:, NI:], xb[:, :NI]], axis=1)
        else:
            xs = xb
        in_maps.append({"x": np.ascontiguousarray(xs.astype(bf)), **shared})
    return in_maps


def _assemble(results):
    y = np.empty((B, C, N), np.float32)
    for core in range(NCORES):
        bi, half = divmod(core, 2)
        y[bi][:, half * NI : (half + 1) * NI] = results[core]["out"]
    return y.reshape(B, C, H, W)


def kernel(x, gn_w, gn_b, qkv_w, qkv_b, proj_w, proj_b):
    from concourse.bass_utils import run_bass_kernel_spmd

    if "nc" not in _CACHE:
        _CACHE["nc"] = _build_nc()
    nc = _CACHE["nc"]
    in_maps = _prep_in_maps(x, gn_w, gn_b, qkv_w, qkv_b, proj_w, proj_b)
    res = run_bass_kernel_spmd(nc, in_maps, core_ids=list(range(NCORES)))
    return _assemble(res.results)


# revision 24
# speedup vs baseline: 1.0096x; 1.0096x over previous
"""AttentionBlock (GroupNorm -> 1x1 qkv -> 4-head attention over 64x64 -> proj -> residual)
distributed over 8 Trainium2 NeuronCores.

Sharding: 8 shards = batch(4) x query-half(2). Each core receives the full
[256, 4096] feature map of its batch element (columns rolled so its own query
half is always columns 0:2048 -> identical SPMD graph on every core).

Key optimizations over the bf16 baseline (366 us):
- GroupNorm folded into runtime-scaled qkv weights (W' = a_c * W, bias via tiny
  PE matvecs); x cast to bf16 once, no normalized-x materialization.
- The whole attention inner loop runs in fp8e4 DoubleRow matmuls (2x contraction
  per column): scores contract q against both k head-planes with a zero-plane
  masking trick (q8 planes [q0,q1,0,q2,q3]; head h reads plane pair (h-plane,
  zero-plane) so the unwanted head contributes nothing), and attn@v contracts
  two 128-key j-chunks per matmul. K>=65 tiles keep the PE in full-rate 128-row
  mode (K<=64 streams at half rate on TRN2).
- exp(scores) split between Scalar (exact Exp -> fp8, table-free) and Vector
  (Schraudolph: scores arrive pre-scaled by 11.5416 = 8/ln2 via the host q
  scale, so max(psum + c0, 0) cast to uint8 IS the fp8 bit pattern of
  e^(s - DELTA)). GPSIMD cannot read PSUM, so it handles SBUF-only work.
- softmax denominator via a ones column in the fp8 v^T (row 64 of the DoubleRow
  pv matmul), normalization off the critical path via a DRAM-broadcast round
  trip, v-bias and all GroupNorm bias terms pushed into the proj bias.
"""

import sys

sys.path.insert(0, "/opt/trn_rl_repo")

import numpy as np
import ml_dtypes

import concourse.bass as bass
import concourse.tile as tile
from concourse import bacc, mybir

# Problem geometry (hardcoded per harness contract)
B, C, H, W = 4, 256, 64, 64
N = H * W              # 4096 spatial positions
HEADS = 4
D = C // HEADS         # 64
GROUPS = 8
EPS = 1e-5
NCORES = 8
NI = N // 2            # 2048 queries per core
IB = 1024              # i-block
JC = 128               # j-chunk (keys per scores matmul)
NJ = N // JC           # 32 j-chunks
NJP = NJ // 2          # 16 j-chunk pairs (DoubleRow contracts a pair)

DELTA = 2.0                       # score shift: exp(s - DELTA), cancels in softmax
SCHR_A = 8.0 / float(np.log(2))   # 11.5416 = fp8e4 bits per e-fold
SCHR_C0 = 56.3                    # bits = A*(s - DELTA) + C0

F32 = mybir.dt.float32
BF16 = mybir.dt.bfloat16
F8 = mybir.dt.float8e4
U8 = mybir.dt.uint8
DR = mybir.MatmulPerfMode.DoubleRow

# q8 planes: [q0, q1, zero, q2, q3]; head h -> (first plane, plane stride)
Q_PLANES = [(0, 2), (1, 1), (2, 1), (2, 2)]

_CACHE = {}


def _build_nc():
    nc = bacc.Bacc("TRN2", target_bir_lowering=False, debug=False,
                   num_devices=NCORES)

    x_ext = nc.declare_dram_parameter("x", [C, N], BF16, isOutput=False)
    # columns: [ (A/8)*qT (256) | kT (256) | 16*vT (256) ],  A = 8/ln2
    wqkvT_ext = nc.declare_dram_parameter("wqkvT", [C, 3 * C], BF16, isOutput=False)
    wpT_ext = nc.declare_dram_parameter("wpT", [C, C], BF16, isOutput=False)
    qkb_ext = nc.declare_dram_parameter("qkb", [128, 4], F32, isOutput=False)
    pb_ext = nc.declare_dram_parameter("pb", [128, 2], F32, isOutput=False)
    gnw_ext = nc.declare_dram_parameter("gnw", [128, 2], F32, isOutput=False)
    gnb_ext = nc.declare_dram_parameter("gnb", [128, 2], F32, isOutput=False)
    oneh_ext = nc.declare_dram_parameter("oneh", [C, GROUPS], F32, isOutput=False)
    onehT_ext = nc.declare_dram_parameter("onehT", [GROUPS, C], F32, isOutput=False)
    zz_ext = nc.declare_dram_parameter("zz", [128, N], F8, isOutput=False)
    out_ext = nc.declare_dram_parameter("out", [C, NI], F32, isOutput=True)

    with tile.TileContext(nc) as tc:
        with (
            tc.tile_pool(name="persist", bufs=1) as per,
            tc.tile_pool(name="etp", bufs=8) as etp,
            tc.tile_pool(name="ep", bufs=2) as ep,
            tc.tile_pool(name="yp", bufs=2) as yp,
            tc.tile_pool(name="dp", bufs=2, space="DRAM") as dp,
            tc.tile_pool(name="ps", bufs=3, space="PSUM") as ps,
            tc.tile_pool(name="pv", bufs=2, space="PSUM") as pvp,
        ):
            # ---- persistent SBUF tensors ----
            x_sb = [per.tile([128, N], BF16, tag=f"x{t}", name=f"x{t}") for t in range(2)]
            q8 = per.tile([128, 5, NI], F8, tag="q8")
            k8 = per.tile([128, 2, N], F8, tag="k8")
            # v^T per j-chunk: [j%128, jchunk, head, 0:64 v | 64 ones | 65 zero]
            vton = per.tile([128, NJ, HEADS, 128], F8, tag="vton")
            att_sb = [per.tile([128, NI], BF16, tag=f"att{t}", name=f"att{t}") for t in range(2)]
            wraw_sb = [per.tile([128, 3 * C], BF16, tag=f"wr{t}", name=f"wr{t}") for t in range(2)]
            wsc_sb = [per.tile([128, 3 * C], BF16, tag=f"ws{t}", name=f"ws{t}") for t in range(2)]
            wpT_sb = [per.tile([128, C], BF16, tag=f"wp{t}", name=f"wp{t}") for t in range(2)]
            qkb_sb = per.tile([128, 4], F32, tag="qkb")
            biasqk_sb = per.tile([128, 4], F32, tag="biasqk")
            pb_sb = per.tile([128, 2], F32, tag="pb")
            pbrt_sb = per.tile([128, 2], F32, tag="pbrt")
            gnw_sb = per.tile([128, 2], F32, tag="gnw")
            gnb_sb = per.tile([128, 2], F32, tag="gnb")
            oneh_sb = [per.tile([128, GROUPS], F32, tag=f"oneh{t}", name=f"oneh{t}") for t in range(2)]
            onehT_sb = per.tile([GROUPS, C], F32, tag="onehT")
            eps_sb = per.tile([GROUPS, 1], F32, tag="eps")
            ndel_sb = per.tile([128, 1], F32, tag="ndel")
            ab_sb = [per.tile([128, 2], F32, tag=f"ab{t}", name=f"ab{t}") for t in range(2)]
            bvec_sb = [per.tile([128, 1], BF16, tag=f"bv{t}", name=f"bv{t}") for t in range(2)]
            vx16_sb = [per.tile([128, 1], BF16, tag=f"vx{t}", name=f"vx{t}") for t in range(2)]
            gst_sb = per.tile([GROUPS, 4], F32, tag="gst")

            ones1 = per.tile([1, D], F32, tag="ones1")
            nc.vector.memset(ones1[:], 1.0)
            nc.vector.memset(eps_sb[:], EPS)
            nc.vector.memset(ndel_sb[:], -DELTA)
            nc.vector.memset(vton[:, :, :, D : D + 1], 1.0)
            nc.vector.memset(vton[:, :, :, D + 1 : D + 2], 0.0)

            # ---- x DMA + GroupNorm statistics + bf16 cast ----
            stats = [per.tile([128, 8, 6], F32, tag=f"st{t}", name=f"st{t}") for t in range(2)]
            mv = [per.tile([128, 4], F32, tag=f"mv{t}", name=f"mv{t}") for t in range(2)]
            x_queues = {(0, 0): nc.sync, (0, 1): nc.sync,
                        (0, 2): nc.sync, (0, 3): nc.scalar,
                        (1, 0): nc.gpsimd, (1, 1): nc.gpsimd,
                        (1, 2): nc.gpsimd, (1, 3): nc.scalar}
            for t in range(2):
                cs = slice(t * 128, (t + 1) * 128)
                for ch in range(4):
                    chs = slice(ch * IB, (ch + 1) * IB)
                    x_queues[(t, ch)].dma_start(out=x_sb[t][:, chs], in_=x_ext[cs, chs])
                    for s in range(2):
                        sub = ch * 2 + s
                        nc.vector.bn_stats(
                            out=stats[t][:, sub, :],
                            in_=x_sb[t][:, sub * 512 : (sub + 1) * 512],
                        )
                if t == 1:
                    # weights / zeros / small inputs ride behind x
                    for t2 in range(2):
                        cs2 = slice(t2 * 128, (t2 + 1) * 128)
                        nc.sync.dma_start(out=wraw_sb[t2][:], in_=wqkvT_ext[cs2, :])
                        nc.scalar.dma_start(out=wpT_sb[t2][:], in_=wpT_ext[cs2, :])
                        nc.scalar.dma_start(out=oneh_sb[t2][:], in_=oneh_ext[cs2, :])
                    nc.scalar.dma_start(out=qkb_sb[:], in_=qkb_ext[:])
                    nc.scalar.dma_start(out=pb_sb[:], in_=pb_ext[:])
                    nc.sync.dma_start(out=gnw_sb[:], in_=gnw_ext[:])
                    nc.sync.dma_start(out=gnb_sb[:], in_=gnb_ext[:])
                    nc.scalar.dma_start(out=onehT_sb[:], in_=onehT_ext[:])
                    nc.gpsimd.dma_start(out=k8[:, 1, :], in_=zz_ext[:])
                    nc.gpsimd.dma_start(out=q8[:, 2, :], in_=zz_ext[:, 0:NI])
                    nc.gpsimd.dma_start(out=q8[64:128, 0, :], in_=zz_ext[64:128, 0:NI])
                    nc.gpsimd.dma_start(out=q8[0:64, 1, :], in_=zz_ext[0:64, 0:NI])
                    nc.gpsimd.dma_start(out=q8[64:128, 3, :], in_=zz_ext[64:128, 0:NI])
                    nc.gpsimd.dma_start(out=q8[0:64, 4, :], in_=zz_ext[0:64, 0:NI])
                nc.vector.bn_aggr(out=mv[t][:, 0:2], in_=stats[t][:])
                nc.vector.tensor_copy(mv[t][:, 2:3], mv[t][:, 0:1])
                nc.vector.tensor_mul(mv[t][:, 3:4], mv[t][:, 0:1], mv[t][:, 0:1])
                nc.vector.tensor_add(mv[t][:, 3:4], mv[t][:, 1:2], mv[t][:, 3:4])

            # group means of (mean, E[x^2]): [8, 2]
            gpt = ps.tile([128, 2, 512], F32, tag="ps", name="gn_ps")
            gp = gpt[0:GROUPS, 0, 0:2]
            for t in range(2):
                nc.tensor.matmul(
                    gp, oneh_sb[t][:], mv[t][:, 2:4],
                    start=(t == 0), stop=(t == 1),
                )
            # gst columns: 0=mean_g, 1=rstd_g; scratch 2=var, 3=std
            nc.vector.tensor_copy(gst_sb[:, 0:1], gp[:, 0:1])
            nc.vector.tensor_mul(gst_sb[:, 2:3], gst_sb[:, 0:1], gst_sb[:, 0:1])
            nc.vector.tensor_sub(gst_sb[:, 2:3], gp[:, 1:2], gst_sb[:, 2:3])
            # rstd = exp(-0.5*ln(var+eps)); Ln/Exp share one ACT table set
            nc.scalar.activation(
                out=gst_sb[:, 3:4], in_=gst_sb[:, 2:3],
                func=mybir.ActivationFunctionType.Ln,
                bias=eps_sb[:], scale=1.0,
            )
            nc.vector.tensor_scalar_mul(
                out=gst_sb[:, 3:4], in0=gst_sb[:, 3:4], scalar1=-0.5
            )
            nc.scalar.activation(
                out=gst_sb[:, 1:2], in_=gst_sb[:, 3:4],
                func=mybir.ActivationFunctionType.Exp, scale=1.0,
            )

            # broadcast (mean_g, rstd_g) to channels; a = rstd*gn_w,
            # b = gn_b - mean*a; scale weights: ws = a_c * wraw (on GPSIMD)
            bct = ps.tile([128, 2, 512], F32, tag="ps", name="gn_bc")
            for t in range(2):
                bc = bct[:, t, 0:2]
                nc.tensor.matmul(
                    bc, onehT_sb[:, t * 128 : (t + 1) * 128], gst_sb[:, 0:2],
                    start=True, stop=True,
                )
                nc.vector.tensor_mul(ab_sb[t][:, 0:1], bc[:, 1:2], gnw_sb[:, t : t + 1])
                nc.vector.tensor_mul(ab_sb[t][:, 1:2], bc[:, 0:1], ab_sb[t][:, 0:1])
                nc.vector.tensor_sub(ab_sb[t][:, 1:2], gnb_sb[:, t : t + 1], ab_sb[t][:, 1:2])
                nc.vector.tensor_copy(bvec_sb[t][:], ab_sb[t][:, 1:2])
                if t == 0:
                    nc.scalar.activation(
                        out=wsc_sb[t][:], in_=wraw_sb[t][:],
                        func=mybir.ActivationFunctionType.Copy,
                        scale=ab_sb[t][:, 0:1])
                else:
                    nc.vector.tensor_scalar_mul(
                        out=wsc_sb[t][:], in0=wraw_sb[t][:],
                        scalar1=ab_sb[t][:, 0:1])

            # runtime bias matvecs: bp[:, 0:6] = wraw.T @ b_vec (per o-tile)
            bpt = ps.tile([128, 2, 512], F32, tag="ps", name="bias_mv")
            bp = bpt[:, 0, 0:6]
            for ot in range(6):
                for t in range(2):
                    nc.tensor.matmul(
                        bp[:, ot : ot + 1],
                        wraw_sb[t][:, ot * 128 : (ot + 1) * 128],
                        bvec_sb[t][:],
                        start=(t == 0), stop=(t == 1),
                    )
            nc.vector.tensor_add(biasqk_sb[:], qkb_sb[:], bp[:, 0:4])
            for t in range(2):
                nc.vector.tensor_copy(vx16_sb[t][:], bp[:, 4 + t : 5 + t])
            # pb_rt = pb + (proj_w/16) @ (16 Wv b_vec)
            pp2t = ps.tile([128, 2, 512], F32, tag="ps", name="pb_mv")
            pp2 = pp2t[:, 0, 0:2]
            for ot in range(2):
                for t in range(2):
                    nc.tensor.matmul(
                        pp2[:, ot : ot + 1],
                        wpT_sb[t][:, ot * 128 : (ot + 1) * 128],
                        vx16_sb[t][:],
                        start=(t == 0), stop=(t == 1),
                    )
            nc.vector.tensor_add(pbrt_sb[:], pb_sb[:], pp2[:, 0:2])

            # ---- emission helpers ----
            def q_rhs(h, isl):
                p0, st = Q_PLANES[h]
                base = q8[:, 0, isl]
                return bass.AP(
                    tensor=base.tensor,
                    offset=base.offset + p0 * NI,
                    ap=[list(base.ap[0])] + [[st * NI, 2]] +
                       [list(a) for a in base.ap[1:]],
                )

            def qkv_tiles(ot, blocks=None):
                # ot 0,1 = q o-tiles (head pairs); 2,3 = k o-tiles
                ncols = NI if ot < 2 else N
                wcols = slice(ot * 128, (ot + 1) * 128)
                for nb in (range(ncols // IB) if blocks is None else blocks):
                    pp = ps.tile([128, 2, 512], F32, tag="ps", name=f"qkv{ot}_{nb}")
                    for cc in range(2):
                        for nh in range(2):
                            nsl = slice(nb * IB + nh * 512, nb * IB + (nh + 1) * 512)
                            nc.tensor.matmul(
                                pp[:, nh, :], wsc_sb[cc][:, wcols], x_sb[cc][:, nsl],
                                start=(cc == 0), stop=(cc == 1),
                            )
                    nbsl = slice(nb * IB, (nb + 1) * IB)
                    ppf = pp[:].rearrange("p a b -> p (a b)")
                    if ot >= 2:
                        nc.scalar.activation(
                            out=k8[:, ot - 2, nbsl], in_=ppf,
                            func=mybir.ActivationFunctionType.Identity,
                            scale=1.0, bias=biasqk_sb[:, ot : ot + 1],
                        )
                    else:
                        # heads 2*ot (rows 0:64) and 2*ot+1 (rows 64:128) go to
                        # their own q8 planes
                        pl0 = [0, 3][ot]
                        pl1 = [1, 4][ot]
                        nc.vector.tensor_scalar_add(
                            out=q8[0:64, pl0, nbsl], in0=ppf[0:64],
                            scalar1=biasqk_sb[0:64, ot : ot + 1],
                        )
                        nc.vector.tensor_scalar_add(
                            out=q8[64:128, pl1, nbsl], in0=ppf[64:128],
                            scalar1=biasqk_sb[64:128, ot : ot + 1],
                        )

            def vt_pair(jp2):
                # v^T for j-chunks (2*jp2, 2*jp2+1): two [j128, 256] matmul
                # groups -> one fp8 copy
                pj = ps.tile([128, 2, 512], F32, tag="ps", name=f"vt{jp2}")
                for jc in range(2):
                    jsl = slice((2 * jp2 + jc) * JC, (2 * jp2 + jc + 1) * JC)
                    for cc in range(2):
                        nc.tensor.matmul(
                            pj[:, jc, 0:256], x_sb[cc][:, jsl],
                            wsc_sb[cc][:, 512:768],
                            start=(cc == 0), stop=(cc == 1),
                        )
                nc.scalar.activation(
                    out=vton[:, 2 * jp2 : 2 * jp2 + 2, :, 0:D],
                    in_=pj[:, :, 0:256].rearrange("p a (h d) -> p a h d", h=HEADS),
                    func=mybir.ActivationFunctionType.Copy, scale=1.0,
                )

            # exp rotation: ACT exact exp -> fp8; DVE Schraudolph (psum is
            # A*s'; the fp8 bits of e^(s'-DELTA) are max(psum+c0, 0) cast to
            # uint8 -- saturating on hw, max() keeps the sim's wrapping cast
            # safe too). 18:14 ratio balances measured engine loads.
            exp_rota = ([True, False] * 14) + [True] * 4
            exp_rota_early = [True, False]
            exp_ctr = [0]
            exp_early = [True]

            def emit_exp(et, scp):
                rota = exp_rota_early if exp_early[0] else exp_rota
                on_act = rota[exp_ctr[0] % len(rota)]
                exp_ctr[0] += 1
                if on_act:
                    nc.scalar.activation(
                        out=et[:], in_=scp[:],
                        func=mybir.ActivationFunctionType.Exp,
                        scale=1.0 / SCHR_A, bias=ndel_sb[:],
                    )
                else:
                    nc.vector.tensor_scalar(
                        out=et[:].bitcast(U8), in0=scp[:],
                        scalar1=SCHR_C0 - SCHR_A * DELTA, scalar2=0.0,
                        op0=mybir.AluOpType.add, op1=mybir.AluOpType.max,
                    )

            def attn_head(ib, h, with_vt, mid_cb=None, post_ic=None,
                          defer_last=False):
                ht = h // 2
                prow = slice((h % 2) * D, (h % 2) * D + D)
                deferred = []
                for ic in range(2):
                    isl = slice(ib * IB + ic * 512, ib * IB + (ic + 1) * 512)
                    pv = pvp.tile([D + 2, 512], F32, tag="pv",
                                  name=f"pv{ib}_{h}_{ic}")
                    for jp in range(NJP):
                        scp = ps.tile([128, 2, 512], F32, tag="ps",
                                      name=f"sc{ib}_{h}_{jp}_{ic}")
                        et = etp.tile([128, 2, 512], F8, tag="et",
                                      name=f"et{ib}_{h}_{jp}_{ic}")
                        for jc in range(2):
                            j = 2 * jp + jc
                            jsl = slice(j * JC, (j + 1) * JC)
                            nc.tensor.matmul(
                                scp[:, jc, :], k8[:, :, jsl], q_rhs(h, isl),
                                start=True, stop=True, perf_mode=DR,
                            )
                        emit_exp(et, scp)
                        if with_vt and ic == 0 and 3 <= jp <= 14:
                            vt_pair(jp + 1)
                        if mid_cb is not None:
                            mid_cb(jp, ic)
                        nc.tensor.matmul(
                            pv[:], vton[:, 2 * jp : 2 * jp + 2, h, 0 : D + 2],
                            et[:],
                            start=(jp == 0), stop=(jp == NJP - 1),
                            perf_mode=DR,
                        )
                    def epilogue(ic=ic, pv=pv):
                        _epilogue(ib, h, ht, prow, ic, pv, post_ic)
                    if defer_last and ic == 1:
                        deferred.append(epilogue)
                    else:
                        epilogue()
                return deferred

            def _epilogue(ib, h, ht, prow, ic, pv, post_ic):
                    isl = slice(ib * IB + ic * 512, ib * IB + (ic + 1) * 512)
                    # per-half epilogue: normalize off the critical path.
                    # h<3: den broadcast across 64 partitions via a DRAM round
                    # trip. h==3 (proj waits on it): recip the den row in SBUF
                    # and broadcast through the PE with a ones column instead.
                    if h == 3:
                        denr = ep.tile([1, 512], F32, tag="denr",
                                       name=f"denr{ib}_{h}_{ic}")
                        nc.scalar.activation(
                            out=denr[:], in_=pv[D : D + 1, :],
                            func=mybir.ActivationFunctionType.Copy, scale=1.0)
                        nc.vector.reciprocal_approx_fast(out=denr[:], in_=denr[:])
                        rbp = ps.tile([128, 2, 512], F32, tag="ps",
                                      name=f"rbp{ib}_{h}_{ic}")
                        nc.tensor.matmul(rbp[0:D, 0, :], ones1[:], denr[:],
                                         start=True, stop=True)
                        pvs3 = ep.tile([D, 512], F32, tag="pvs3",
                                       name=f"pvs3{ib}_{h}_{ic}")
                        nc.vector.tensor_copy(pvs3[:], pv[0:D, :])
                        nc.vector.tensor_mul(att_sb[ht][prow, isl],
                                             pvs3[:], rbp[0:D, 0, :])
                    else:
                        pvs = ep.tile([D + 2, 512], F32, tag="pvs",
                                      name=f"pvs{ib}_{h}_{ic}")
                        if ic == 0:
                            nc.scalar.activation(
                                out=pvs[:], in_=pv[:],
                                func=mybir.ActivationFunctionType.Copy, scale=1.0)
                        else:
                            nc.vector.tensor_copy(pvs[:], pv[:])
                        dent = dp.tile([1, 512], F32, tag="dent",
                                       name=f"den{ib}_{h}_{ic}")
                        nc.gpsimd.dma_start(out=dent[:], in_=pvs[D : D + 1, :])
                        rbs = ep.tile([D, 512], F32, tag="rbs",
                                      name=f"rbs{ib}_{h}_{ic}")
                        dbc = bass.AP(
                            tensor=dent.tensor, offset=dent.offset,
                            ap=[[0, D]] + [list(a) for a in dent.ap[1:]],
                        )
                        nc.gpsimd.dma_start(out=rbs[:], in_=dbc)
                        nc.vector.reciprocal_approx_fast(out=rbs[:], in_=rbs[:])
                        nc.gpsimd.tensor_mul(att_sb[ht][prow, isl],
                                             pvs[0:D, :], rbs[:])
                    if post_ic is not None:
                        post_ic(ic)

            def proj_part(ib, cc, ypart_tiles, halves=(0, 1), final=False):
                for ot in range(2):
                    pp = ps.tile([128, 2, 512], F32, tag="ps",
                                 name=f"pj{ib}_{cc}_{ot}_{halves[0]}")
                    wcols = slice(ot * 128, (ot + 1) * 128)
                    for nh in halves:
                        asl = slice(ib * IB + nh * 512, ib * IB + (nh + 1) * 512)
                        nc.tensor.matmul(
                            pp[:, nh, :], wpT_sb[cc][:, wcols], att_sb[cc][:, asl],
                            start=True, stop=True,
                        )
                    for nh in halves:
                        psl = pp[:, nh, :]
                        ysl = slice(ib * IB + nh * 512, ib * IB + (nh + 1) * 512)
                        csl = slice(nh * 512, (nh + 1) * 512)
                        if cc == 0:
                            if nh == halves[0]:
                                yt = yp.tile([128, IB], F32, tag=f"ypart{ot}",
                                             name=f"ypart{ib}_{ot}")
                                ypart_tiles.append(yt)
                            yt = ypart_tiles[ot]
                            nc.scalar.activation(
                                out=yt[:, csl], in_=psl,
                                func=mybir.ActivationFunctionType.Identity,
                                scale=1.0, bias=pbrt_sb[:, ot : ot + 1])
                        else:
                            y_sb = yp.tile([128, 512], F32, tag="y",
                                           name=f"y{ib}_{ot}_{nh}")
                            add1 = nc.vector if final else nc.gpsimd
                            add1.tensor_add(y_sb[:], ypart_tiles[ot][:, csl],
                                            x_sb[ot][:, ysl])
                            nc.vector.tensor_add(y_sb[:], y_sb[:], psl)
                            nc.sync.dma_start(
                                out=out_ext[ot * 128 : (ot + 1) * 128, ysl],
                                in_=y_sb[:])

            # ---- schedule ----
            qkv_tiles(0)        # q heads 0,1 (q8 planes 0,1)
            for jp2 in range(4):
                vt_pair(jp2)    # v chunks for jp 0..3
            qkv_tiles(2, [0])   # k heads 0,1, first block
            yparts = {}
            pending_ep = []
            for ib in range(NI // IB):
                yparts[ib] = []
                for h in range(HEADS):
                    if ib == 0 and h == 0:
                        # k plane0 blocks 1-3 stream in ahead of first use
                        mid = (lambda jp, ic: qkv_tiles(2, [1 + jp // 4])
                               if (ic == 0 and jp in (0, 4, 8)) else None)
                    elif ib == 0 and h == 1:
                        # q planes 3,4 and k plane1 before heads 2,3
                        def mid(jp, ic):
                            if ic == 0 and jp in (0, 4, 8):
                                qkv_tiles(3, [1 + jp // 4] if jp else [0, 1])
                            elif ic == 1 and jp in (0, 8):
                                qkv_tiles(1, [jp // 8])
                    elif ib > 0 and h == 0:
                        mid = (lambda jp, ic, p=ib - 1: proj_part(p, 1, yparts[p])
                               if (jp == 4 and ic == 0) else None)
                    elif h == 3:
                        mid = (lambda jp, ic, p=ib: proj_part(p, 0, yparts[p])
                               if (jp == 4 and ic == 0) else None)
                    else:
                        mid = None
                    last = (ib == NI // IB - 1 and h == 3)
                    if last:
                        prev_mid = mid
                        def mid(jp, ic, pm=prev_mid, p=ib):
                            if pm is not None:
                                pm(jp, ic)
                            if ic == 1 and jp == 8:
                                proj_part(p, 1, yparts[p], halves=(0,), final=True)
                        post = (lambda ic, p=ib: proj_part(
                            p, 1, yparts[p], halves=(1,), final=True)
                            if ic == 1 else None)
                    else:
                        post = None
                    prev2 = mid
                    def mid(jp, ic, pm=prev2, eps_=tuple(pending_ep)):
                        if ic == 0 and jp == 2:
                            for e in eps_:
                                e()
                        if pm is not None:
                            pm(jp, ic)
                    pending_ep = attn_head(
                        ib, h, with_vt=(ib == 0 and h == 0), mid_cb=mid,
                        post_ic=post, defer_last=not last)
                    if ib == 0 and h == 1:
                        exp_early[0] = False

    nc.compile()
    return nc


def _prep_in_maps(x, gn_w, gn_b, qkv_w, qkv_b, proj_w, proj_b):
    x = np.ascontiguousarray(np.asarray(x, np.float32)).reshape(B, C, N)
    qkv_w = np.asarray(qkv_w, np.float32)
    qkv_b = np.asarray(qkv_b, np.float32)
    proj_w = np.asarray(proj_w, np.float32)
    proj_b = np.asarray(proj_b, np.float32)
    gn_w = np.asarray(gn_w, np.float32)
    gn_b = np.asarray(gn_b, np.float32)

    bf = ml_dtypes.bfloat16
    qs = SCHR_A / 8.0             # fold D^-0.5 and the Schraudolph slope into q
    wq = qkv_w[:C] * qs
    wk = qkv_w[C : 2 * C]
    wv = 16.0 * qkv_w[2 * C :]    # scale v for fp8; /16 folded into wpT
    wqkvT = np.ascontiguousarray(np.concatenate([wq.T, wk.T, wv.T], axis=1)).astype(bf)
    wpT = np.ascontiguousarray(proj_w.T / 16.0).astype(bf)
    qkb = np.ascontiguousarray(
        np.concatenate([(qkv_b[:C] * qs).reshape(2, 128).T,
                        qkv_b[C : 2 * C].reshape(2, 128).T], axis=1))
    # fold v-bias through proj: proj(att + vb) = proj(att) + proj_w @ vb
    pb_eff = proj_b + proj_w.astype(np.float64) @ qkv_b[2 * C :].astype(np.float64)
    pb = np.ascontiguousarray(pb_eff.astype(np.float32).reshape(2, 128).T)
    gnw2 = np.ascontiguousarray(gn_w.reshape(2, 128).T)
    gnb2 = np.ascontiguousarray(gn_b.reshape(2, 128).T)
    cidx = np.arange(C)
    oneh = (cidx[:, None] // 32 == np.arange(GROUPS)[None, :]).astype(np.float32) / 32.0
    onehT = np.ascontiguousarray(oneh.T * 32.0)
    zz = np.zeros((128, N), ml_dtypes.float8_e4m3)

    shared = {
        "wqkvT": wqkvT, "wpT": wpT, "qkb": qkb, "pb": pb,
        "gnw": gnw2, "gnb": gnb2, "oneh": oneh, "onehT": onehT, "zz": zz,
    }
    in_maps = []
    for core in range(NCORES):
        bi, half = divmod(core, 2)
        xb = x[bi]
        if half:
            xs = np.concatenate([xb[:, NI:], xb[:, :NI]], axis=1)
        else:
            xs = xb
        in_maps.append({"x": np.ascontiguousarray(xs.astype(bf)), **shared})
    return in_maps


def _assemble(results):
    y = np.empty((B, C, N), np.float32)
    for core in range(NCORES):
        bi, half = divmod(core, 2)
        y[bi][:, half * NI : (half + 1) * NI] = results[core]["out"]
    return y.reshape(B, C, H, W)


def kernel(x, gn_w, gn_b, qkv_w, qkv_b, proj_w, proj_b):
    from concourse.bass_utils import run_bass_kernel_spmd

    if "nc" not in _CACHE:
        _CACHE["nc"] = _build_nc()
    nc = _CACHE["nc"]
    in_maps = _prep_in_maps(x, gn_w, gn_b, qkv_w, qkv_b, proj_w, proj_b)
    res = run_bass_kernel_spmd(nc, in_maps, core_ids=list(range(NCORES)))
    return _assemble(res.results)


# revision 25
# speedup vs baseline: 1.0107x; 1.0011x over previous
"""AttentionBlock (GroupNorm -> 1x1 qkv -> 4-head attention over 64x64 -> proj -> residual)
distributed over 8 Trainium2 NeuronCores.

Sharding: 8 shards = batch(4) x query-half(2). Each core receives the full
[256, 4096] feature map of its batch element (columns rolled so its own query
half is always columns 0:2048 -> identical SPMD graph on every core).

Key optimizations over the bf16 baseline (366 us):
- GroupNorm folded into runtime-scaled qkv weights (W' = a_c * W, bias via tiny
  PE matvecs); x cast to bf16 once, no normalized-x materialization.
- The whole attention inner loop runs in fp8e4 DoubleRow matmuls (2x contraction
  per column): scores contract q against both k head-planes with a zero-plane
  masking trick (q8 planes [q0,q1,0,q2,q3]; head h reads plane pair (h-plane,
  zero-plane) so the unwanted head contributes nothing), and attn@v contracts
  two 128-key j-chunks per matmul. K>=65 tiles keep the PE in full-rate 128-row
  mode (K<=64 streams at half rate on TRN2).
- exp(scores) split between Scalar (exact Exp -> fp8, table-free) and Vector
  (Schraudolph: scores arrive pre-scaled by 11.5416 = 8/ln2 via the host q
  scale, so max(psum + c0, 0) cast to uint8 IS the fp8 bit pattern of
  e^(s - DELTA)). GPSIMD cannot read PSUM, so it handles SBUF-only work.
- softmax denominator via a ones column in the fp8 v^T (row 64 of the DoubleRow
  pv matmul), normalization off the critical path via a DRAM-broadcast round
  trip, v-bias and all GroupNorm bias terms pushed into the proj bias.
"""

import sys

sys.path.insert(0, "/opt/trn_rl_repo")

import numpy as np
import ml_dtypes

import concourse.bass as bass
import concourse.tile as tile
from concourse import bacc, mybir

# Problem geometry (hardcoded per harness contract)
B, C, H, W = 4, 256, 64, 64
N = H * W              # 4096 spatial positions
HEADS = 4
D = C // HEADS         # 64
GROUPS = 8
EPS = 1e-5
NCORES = 8
NI = N // 2            # 2048 queries per core
IB = 1024              # i-block
JC = 128               # j-chunk (keys per scores matmul)
NJ = N // JC           # 32 j-chunks
NJP = NJ // 2          # 16 j-chunk pairs (DoubleRow contracts a pair)

DELTA = 2.0                       # score shift: exp(s - DELTA), cancels in softmax
SCHR_A = 8.0 / float(np.log(2))   # 11.5416 = fp8e4 bits per e-fold
SCHR_C0 = 56.3                    # bits = A*(s - DELTA) + C0

F32 = mybir.dt.float32
BF16 = mybir.dt.bfloat16
F8 = mybir.dt.float8e4
U8 = mybir.dt.uint8
DR = mybir.MatmulPerfMode.DoubleRow

# q8 planes: [q0, q1, zero, q2, q3]; head h -> (first plane, plane stride)
Q_PLANES = [(0, 2), (1, 1), (2, 1), (2, 2)]

_CACHE = {}


def _build_nc():
    nc = bacc.Bacc("TRN2", target_bir_lowering=False, debug=False,
                   num_devices=NCORES)

    x_ext = nc.declare_dram_parameter("x", [C, N], BF16, isOutput=False)
    # columns: [ (A/8)*qT (256) | kT (256) | 16*vT (256) ],  A = 8/ln2
    wqkvT_ext = nc.declare_dram_parameter("wqkvT", [C, 3 * C], BF16, isOutput=False)
    wpT_ext = nc.declare_dram_parameter("wpT", [C, C], BF16, isOutput=False)
    qkb_ext = nc.declare_dram_parameter("qkb", [128, 4], F32, isOutput=False)
    pb_ext = nc.declare_dram_parameter("pb", [128, 2], F32, isOutput=False)
    gnw_ext = nc.declare_dram_parameter("gnw", [128, 2], F32, isOutput=False)
    gnb_ext = nc.declare_dram_parameter("gnb", [128, 2], F32, isOutput=False)
    oneh_ext = nc.declare_dram_parameter("oneh", [C, GROUPS], F32, isOutput=False)
    onehT_ext = nc.declare_dram_parameter("onehT", [GROUPS, C], F32, isOutput=False)
    zz_ext = nc.declare_dram_parameter("zz", [128, N], F8, isOutput=False)
    out_ext = nc.declare_dram_parameter("out", [C, NI], F32, isOutput=True)

    with tile.TileContext(nc) as tc:
        with (
            tc.tile_pool(name="persist", bufs=1) as per,
            tc.tile_pool(name="etp", bufs=6) as etp,
            tc.tile_pool(name="ep", bufs=2) as ep,
            tc.tile_pool(name="yp", bufs=2) as yp,
            tc.tile_pool(name="dp", bufs=2, space="DRAM") as dp,
            tc.tile_pool(name="ps", bufs=3, space="PSUM") as ps,
            tc.tile_pool(name="pv", bufs=2, space="PSUM") as pvp,
        ):
            # ---- persistent SBUF tensors ----
            x_sb = [per.tile([128, N], BF16, tag=f"x{t}", name=f"x{t}") for t in range(2)]
            q8 = per.tile([128, 5, NI], F8, tag="q8")
            k8 = per.tile([128, 2, N], F8, tag="k8")
            # v^T per j-chunk: [j%128, jchunk, head, 0:64 v | 64 ones | 65 zero]
            vton = per.tile([128, NJ, HEADS, 128], F8, tag="vton")
            att_sb = [per.tile([128, NI], BF16, tag=f"att{t}", name=f"att{t}") for t in range(2)]
            wraw_sb = [per.tile([128, 3 * C], BF16, tag=f"wr{t}", name=f"wr{t}") for t in range(2)]
            wsc_sb = [per.tile([128, 3 * C], BF16, tag=f"ws{t}", name=f"ws{t}") for t in range(2)]
            wpT_sb = [per.tile([128, C], BF16, tag=f"wp{t}", name=f"wp{t}") for t in range(2)]
            qkb_sb = per.tile([128, 4], F32, tag="qkb")
            biasqk_sb = per.tile([128, 4], F32, tag="biasqk")
            pb_sb = per.tile([128, 2], F32, tag="pb")
            pbrt_sb = per.tile([128, 2], F32, tag="pbrt")
            gnw_sb = per.tile([128, 2], F32, tag="gnw")
            gnb_sb = per.tile([128, 2], F32, tag="gnb")
            oneh_sb = [per.tile([128, GROUPS], F32, tag=f"oneh{t}", name=f"oneh{t}") for t in range(2)]
            onehT_sb = per.tile([GROUPS, C], F32, tag="onehT")
            eps_sb = per.tile([GROUPS, 1], F32, tag="eps")
            ndel_sb = per.tile([128, 1], F32, tag="ndel")
            ab_sb = [per.tile([128, 2], F32, tag=f"ab{t}", name=f"ab{t}") for t in range(2)]
            bvec_sb = [per.tile([128, 1], BF16, tag=f"bv{t}", name=f"bv{t}") for t in range(2)]
            vx16_sb = [per.tile([128, 1], BF16, tag=f"vx{t}", name=f"vx{t}") for t in range(2)]
            gst_sb = per.tile([GROUPS, 4], F32, tag="gst")

            ones1 = per.tile([1, D], F32, tag="ones1")
            nc.vector.memset(ones1[:], 1.0)
            nc.vector.memset(eps_sb[:], EPS)
            nc.vector.memset(ndel_sb[:], -DELTA)
            nc.vector.memset(vton[:, :, :, D : D + 1], 1.0)
            nc.vector.memset(vton[:, :, :, D + 1 : D + 2], 0.0)

            # ---- x DMA + GroupNorm statistics + bf16 cast ----
            stats = [per.tile([128, 8, 6], F32, tag=f"st{t}", name=f"st{t}") for t in range(2)]
            mv = [per.tile([128, 4], F32, tag=f"mv{t}", name=f"mv{t}") for t in range(2)]
            x_queues = {(0, 0): nc.sync, (0, 1): nc.sync,
                        (0, 2): nc.sync, (0, 3): nc.scalar,
                        (1, 0): nc.gpsimd, (1, 1): nc.gpsimd,
                        (1, 2): nc.gpsimd, (1, 3): nc.scalar}
            for t in range(2):
                cs = slice(t * 128, (t + 1) * 128)
                for ch in range(4):
                    chs = slice(ch * IB, (ch + 1) * IB)
                    x_queues[(t, ch)].dma_start(out=x_sb[t][:, chs], in_=x_ext[cs, chs])
                    for s in range(2):
                        sub = ch * 2 + s
                        nc.vector.bn_stats(
                            out=stats[t][:, sub, :],
                            in_=x_sb[t][:, sub * 512 : (sub + 1) * 512],
                        )
                if t == 1:
                    # weights / zeros / small inputs ride behind x
                    for t2 in range(2):
                        cs2 = slice(t2 * 128, (t2 + 1) * 128)
                        nc.sync.dma_start(out=wraw_sb[t2][:], in_=wqkvT_ext[cs2, :])
                        nc.scalar.dma_start(out=wpT_sb[t2][:], in_=wpT_ext[cs2, :])
                        nc.scalar.dma_start(out=oneh_sb[t2][:], in_=oneh_ext[cs2, :])
                    nc.scalar.dma_start(out=qkb_sb[:], in_=qkb_ext[:])
                    nc.scalar.dma_start(out=pb_sb[:], in_=pb_ext[:])
                    nc.sync.dma_start(out=gnw_sb[:], in_=gnw_ext[:])
                    nc.sync.dma_start(out=gnb_sb[:], in_=gnb_ext[:])
                    nc.scalar.dma_start(out=onehT_sb[:], in_=onehT_ext[:])
                    nc.gpsimd.dma_start(out=k8[:, 1, :], in_=zz_ext[:])
                    nc.gpsimd.dma_start(out=q8[:, 2, :], in_=zz_ext[:, 0:NI])
                    nc.gpsimd.dma_start(out=q8[64:128, 0, :], in_=zz_ext[64:128, 0:NI])
                    nc.gpsimd.dma_start(out=q8[0:64, 1, :], in_=zz_ext[0:64, 0:NI])
                    nc.gpsimd.dma_start(out=q8[64:128, 3, :], in_=zz_ext[64:128, 0:NI])
                    nc.gpsimd.dma_start(out=q8[0:64, 4, :], in_=zz_ext[0:64, 0:NI])
                nc.vector.bn_aggr(out=mv[t][:, 0:2], in_=stats[t][:])
                nc.vector.tensor_copy(mv[t][:, 2:3], mv[t][:, 0:1])
                nc.vector.tensor_mul(mv[t][:, 3:4], mv[t][:, 0:1], mv[t][:, 0:1])
                nc.vector.tensor_add(mv[t][:, 3:4], mv[t][:, 1:2], mv[t][:, 3:4])

            # group means of (mean, E[x^2]): [8, 2]
            gpt = ps.tile([128, 2, 512], F32, tag="ps", name="gn_ps")
            gp = gpt[0:GROUPS, 0, 0:2]
            for t in range(2):
                nc.tensor.matmul(
                    gp, oneh_sb[t][:], mv[t][:, 2:4],
                    start=(t == 0), stop=(t == 1),
                )
            # gst columns: 0=mean_g, 1=rstd_g; scratch 2=var, 3=std
            nc.vector.tensor_copy(gst_sb[:, 0:1], gp[:, 0:1])
            nc.vector.tensor_mul(gst_sb[:, 2:3], gst_sb[:, 0:1], gst_sb[:, 0:1])
            nc.vector.tensor_sub(gst_sb[:, 2:3], gp[:, 1:2], gst_sb[:, 2:3])
            # rstd = exp(-0.5*ln(var+eps)); Ln/Exp share one ACT table set
            nc.scalar.activation(
                out=gst_sb[:, 3:4], in_=gst_sb[:, 2:3],
                func=mybir.ActivationFunctionType.Ln,
                bias=eps_sb[:], scale=1.0,
            )
            nc.vector.tensor_scalar_mul(
                out=gst_sb[:, 3:4], in0=gst_sb[:, 3:4], scalar1=-0.5
            )
            nc.scalar.activation(
                out=gst_sb[:, 1:2], in_=gst_sb[:, 3:4],
                func=mybir.ActivationFunctionType.Exp, scale=1.0,
            )

            # broadcast (mean_g, rstd_g) to channels; a = rstd*gn_w,
            # b = gn_b - mean*a; scale weights: ws = a_c * wraw (on GPSIMD)
            bct = ps.tile([128, 2, 512], F32, tag="ps", name="gn_bc")
            for t in range(2):
                bc = bct[:, t, 0:2]
                nc.tensor.matmul(
                    bc, onehT_sb[:, t * 128 : (t + 1) * 128], gst_sb[:, 0:2],
                    start=True, stop=True,
                )
                nc.vector.tensor_mul(ab_sb[t][:, 0:1], bc[:, 1:2], gnw_sb[:, t : t + 1])
                nc.vector.tensor_mul(ab_sb[t][:, 1:2], bc[:, 0:1], ab_sb[t][:, 0:1])
                nc.vector.tensor_sub(ab_sb[t][:, 1:2], gnb_sb[:, t : t + 1], ab_sb[t][:, 1:2])
                nc.vector.tensor_copy(bvec_sb[t][:], ab_sb[t][:, 1:2])
                nc.vector.tensor_scalar_mul(
                    out=wsc_sb[t][:], in0=wraw_sb[t][:], scalar1=ab_sb[t][:, 0:1]
                )

            # runtime bias matvecs: bp[:, 0:6] = wraw.T @ b_vec (per o-tile)
            bpt = ps.tile([128, 2, 512], F32, tag="ps", name="bias_mv")
            bp = bpt[:, 0, 0:6]
            for ot in range(6):
                for t in range(2):
                    nc.tensor.matmul(
                        bp[:, ot : ot + 1],
                        wraw_sb[t][:, ot * 128 : (ot + 1) * 128],
                        bvec_sb[t][:],
                        start=(t == 0), stop=(t == 1),
                    )
            nc.vector.tensor_add(biasqk_sb[:], qkb_sb[:], bp[:, 0:4])
            for t in range(2):
                nc.vector.tensor_copy(vx16_sb[t][:], bp[:, 4 + t : 5 + t])
            # pb_rt = pb + (proj_w/16) @ (16 Wv b_vec)
            pp2t = ps.tile([128, 2, 512], F32, tag="ps", name="pb_mv")
            pp2 = pp2t[:, 0, 0:2]
            for ot in range(2):
                for t in range(2):
                    nc.tensor.matmul(
                        pp2[:, ot : ot + 1],
                        wpT_sb[t][:, ot * 128 : (ot + 1) * 128],
                        vx16_sb[t][:],
                        start=(t == 0), stop=(t == 1),
                    )
            nc.vector.tensor_add(pbrt_sb[:], pb_sb[:], pp2[:, 0:2])

            # ---- emission helpers ----
            def q_rhs(h, isl):
                p0, st = Q_PLANES[h]
                base = q8[:, 0, isl]
                return bass.AP(
                    tensor=base.tensor,
                    offset=base.offset + p0 * NI,
                    ap=[list(base.ap[0])] + [[st * NI, 2]] +
                       [list(a) for a in base.ap[1:]],
                )

            def qkv_tiles(ot, blocks=None):
                # ot 0,1 = q o-tiles (head pairs); 2,3 = k o-tiles
                ncols = NI if ot < 2 else N
                wcols = slice(ot * 128, (ot + 1) * 128)
                for nb in (range(ncols // IB) if blocks is None else blocks):
                    pp = ps.tile([128, 2, 512], F32, tag="ps", name=f"qkv{ot}_{nb}")
                    for cc in range(2):
                        for nh in range(2):
                            nsl = slice(nb * IB + nh * 512, nb * IB + (nh + 1) * 512)
                            nc.tensor.matmul(
                                pp[:, nh, :], wsc_sb[cc][:, wcols], x_sb[cc][:, nsl],
                                start=(cc == 0), stop=(cc == 1),
                            )
                    nbsl = slice(nb * IB, (nb + 1) * IB)
                    ppf = pp[:].rearrange("p a b -> p (a b)")
                    if ot >= 2:
                        nc.scalar.activation(
                            out=k8[:, ot - 2, nbsl], in_=ppf,
                            func=mybir.ActivationFunctionType.Identity,
                            scale=1.0, bias=biasqk_sb[:, ot : ot + 1],
                        )
                    else:
                        # heads 2*ot (rows 0:64) and 2*ot+1 (rows 64:128) go to
                        # their own q8 planes
                        pl0 = [0, 3][ot]
                        pl1 = [1, 4][ot]
                        nc.vector.tensor_scalar_add(
                            out=q8[0:64, pl0, nbsl], in0=ppf[0:64],
                            scalar1=biasqk_sb[0:64, ot : ot + 1],
                        )
                        nc.vector.tensor_scalar_add(
                            out=q8[64:128, pl1, nbsl], in0=ppf[64:128],
                            scalar1=biasqk_sb[64:128, ot : ot + 1],
                        )

            def vt_pair(jp2):
                # v^T for j-chunks (2*jp2, 2*jp2+1): two [j128, 256] matmul
                # groups -> one fp8 copy
                pj = ps.tile([128, 2, 512], F32, tag="ps", name=f"vt{jp2}")
                for jc in range(2):
                    jsl = slice((2 * jp2 + jc) * JC, (2 * jp2 + jc + 1) * JC)
                    for cc in range(2):
                        nc.tensor.matmul(
                            pj[:, jc, 0:256], x_sb[cc][:, jsl],
                            wsc_sb[cc][:, 512:768],
                            start=(cc == 0), stop=(cc == 1),
                        )
                nc.scalar.activation(
                    out=vton[:, 2 * jp2 : 2 * jp2 + 2, :, 0:D],
                    in_=pj[:, :, 0:256].rearrange("p a (h d) -> p a h d", h=HEADS),
                    func=mybir.ActivationFunctionType.Copy, scale=1.0,
                )

            # exp rotation: ACT exact exp -> fp8; DVE Schraudolph (psum is
            # A*s'; the fp8 bits of e^(s'-DELTA) are max(psum+c0, 0) cast to
            # uint8 -- saturating on hw, max() keeps the sim's wrapping cast
            # safe too). 18:14 ratio balances measured engine loads.
            exp_rota = ([True, False] * 14) + [True] * 4
            exp_rota_early = [True, False]
            exp_ctr = [0]
            exp_early = [True]

            def emit_exp(et, scp):
                rota = exp_rota_early if exp_early[0] else exp_rota
                on_act = rota[exp_ctr[0] % len(rota)]
                exp_ctr[0] += 1
                if on_act:
                    nc.scalar.activation(
                        out=et[:], in_=scp[:],
                        func=mybir.ActivationFunctionType.Exp,
                        scale=1.0 / SCHR_A, bias=ndel_sb[:],
                    )
                else:
                    nc.vector.tensor_scalar(
                        out=et[:].bitcast(U8), in0=scp[:],
                        scalar1=SCHR_C0 - SCHR_A * DELTA, scalar2=0.0,
                        op0=mybir.AluOpType.add, op1=mybir.AluOpType.max,
                    )

            def attn_head(ib, h, with_vt, mid_cb=None, post_ic=None,
                          defer_last=False):
                ht = h // 2
                prow = slice((h % 2) * D, (h % 2) * D + D)
                deferred = []
                for ic in range(2):
                    isl = slice(ib * IB + ic * 512, ib * IB + (ic + 1) * 512)
                    pv = pvp.tile([D + 2, 512], F32, tag="pv",
                                  name=f"pv{ib}_{h}_{ic}")
                    for jp in range(NJP):
                        scp = ps.tile([128, 2, 512], F32, tag="ps",
                                      name=f"sc{ib}_{h}_{jp}_{ic}")
                        et = etp.tile([128, 2, 512], F8, tag="et",
                                      name=f"et{ib}_{h}_{jp}_{ic}")
                        for jc in range(2):
                            j = 2 * jp + jc
                            jsl = slice(j * JC, (j + 1) * JC)
                            nc.tensor.matmul(
                                scp[:, jc, :], k8[:, :, jsl], q_rhs(h, isl),
                                start=True, stop=True, perf_mode=DR,
                            )
                        emit_exp(et, scp)
                        if with_vt and ic == 0 and 3 <= jp <= 14:
                            vt_pair(jp + 1)
                        if mid_cb is not None:
                            mid_cb(jp, ic)
                        nc.tensor.matmul(
                            pv[:], vton[:, 2 * jp : 2 * jp + 2, h, 0 : D + 2],
                            et[:],
                            start=(jp == 0), stop=(jp == NJP - 1),
                            perf_mode=DR,
                        )
                    def epilogue(ic=ic, pv=pv):
                        _epilogue(ib, h, ht, prow, ic, pv, post_ic)
                    if defer_last and ic == 1:
                        deferred.append(epilogue)
                    else:
                        epilogue()
                return deferred

            def _epilogue(ib, h, ht, prow, ic, pv, post_ic):
                    isl = slice(ib * IB + ic * 512, ib * IB + (ic + 1) * 512)
                    # per-half epilogue: normalize off the critical path.
                    # h<3: den broadcast across 64 partitions via a DRAM round
                    # trip. h==3 (proj waits on it): recip the den row in SBUF
                    # and broadcast through the PE with a ones column instead.
                    if h == 3:
                        denr = ep.tile([1, 512], F32, tag="denr",
                                       name=f"denr{ib}_{h}_{ic}")
                        nc.scalar.activation(
                            out=denr[:], in_=pv[D : D + 1, :],
                            func=mybir.ActivationFunctionType.Copy, scale=1.0)
                        nc.vector.reciprocal_approx_fast(out=denr[:], in_=denr[:])
                        rbp = ps.tile([128, 2, 512], F32, tag="ps",
                                      name=f"rbp{ib}_{h}_{ic}")
                        nc.tensor.matmul(rbp[0:D, 0, :], ones1[:], denr[:],
                                         start=True, stop=True)
                        pvs3 = ep.tile([D, 512], F32, tag="pvs3",
                                       name=f"pvs3{ib}_{h}_{ic}")
                        nc.vector.tensor_copy(pvs3[:], pv[0:D, :])
                        nc.vector.tensor_mul(att_sb[ht][prow, isl],
                                             pvs3[:], rbp[0:D, 0, :])
                    else:
                        pvs = ep.tile([D + 2, 512], F32, tag="pvs",
                                      name=f"pvs{ib}_{h}_{ic}")
                        if ic == 0:
                            nc.scalar.activation(
                                out=pvs[:], in_=pv[:],
                                func=mybir.ActivationFunctionType.Copy, scale=1.0)
                        else:
                            nc.vector.tensor_copy(pvs[:], pv[:])
                        dent = dp.tile([1, 512], F32, tag="dent",
                                       name=f"den{ib}_{h}_{ic}")
                        nc.gpsimd.dma_start(out=dent[:], in_=pvs[D : D + 1, :])
                        rbs = ep.tile([D, 512], F32, tag="rbs",
                                      name=f"rbs{ib}_{h}_{ic}")
                        dbc = bass.AP(
                            tensor=dent.tensor, offset=dent.offset,
                            ap=[[0, D]] + [list(a) for a in dent.ap[1:]],
                        )
                        nc.gpsimd.dma_start(out=rbs[:], in_=dbc)
                        nc.vector.reciprocal_approx_fast(out=rbs[:], in_=rbs[:])
                        nc.gpsimd.tensor_mul(att_sb[ht][prow, isl],
                                             pvs[0:D, :], rbs[:])
                    if post_ic is not None:
                        post_ic(ic)

            def proj_part(ib, cc, ypart_tiles, halves=(0, 1), final=False):
                for ot in range(2):
                    pp = ps.tile([128, 2, 512], F32, tag="ps",
                                 name=f"pj{ib}_{cc}_{ot}_{halves[0]}")
                    wcols = slice(ot * 128, (ot + 1) * 128)
                    for nh in halves:
                        asl = slice(ib * IB + nh * 512, ib * IB + (nh + 1) * 512)
                        nc.tensor.matmul(
                            pp[:, nh, :], wpT_sb[cc][:, wcols], att_sb[cc][:, asl],
                            start=True, stop=True,
                        )
                    for nh in halves:
                        psl = pp[:, nh, :]
                        ysl = slice(ib * IB + nh * 512, ib * IB + (nh + 1) * 512)
                        csl = slice(nh * 512, (nh + 1) * 512)
                        if cc == 0:
                            if nh == halves[0]:
                                yt = yp.tile([128, IB], F32, tag=f"ypart{ot}",
                                             name=f"ypart{ib}_{ot}")
                                ypart_tiles.append(yt)
                            yt = ypart_tiles[ot]
                            nc.scalar.activation(
                                out=yt[:, csl], in_=psl,
                                func=mybir.ActivationFunctionType.Identity,
                                scale=1.0, bias=pbrt_sb[:, ot : ot + 1])
                        else:
                            y_sb = yp.tile([128, 512], F32, tag="y",
                                           name=f"y{ib}_{ot}_{nh}")
                            add1 = nc.vector if final else nc.gpsimd
                            add1.tensor_add(y_sb[:], ypart_tiles[ot][:, csl],
                                            x_sb[ot][:, ysl])
                            nc.vector.tensor_add(y_sb[:], y_sb[:], psl)
                            nc.sync.dma_start(
                                out=out_ext[ot * 128 : (ot + 1) * 128, ysl],
                                in_=y_sb[:])

            # ---- schedule ----
            qkv_tiles(0)        # q heads 0,1 (q8 planes 0,1)
            for jp2 in range(4):
                vt_pair(jp2)    # v chunks for jp 0..3
            qkv_tiles(2, [0])   # k heads 0,1, first block
            yparts = {}
            pending_ep = []
            for ib in range(NI // IB):
                yparts[ib] = []
                for h in range(HEADS):
                    if ib == 0 and h == 0:
                        # k plane0 blocks 1-3 stream in ahead of first use
                        mid = (lambda jp, ic: qkv_tiles(2, [1 + jp // 4])
                               if (ic == 0 and jp in (0, 4, 8)) else None)
                    elif ib == 0 and h == 1:
                        # q planes 3,4 and k plane1 before heads 2,3
                        def mid(jp, ic):
                            if ic == 0 and jp in (0, 4, 8):
                                qkv_tiles(3, [1 + jp // 4] if jp else [0, 1])
                            elif ic == 1 and jp in (0, 8):
                                qkv_tiles(1, [jp // 8])
                    elif ib > 0 and h == 0:
                        mid = (lambda jp, ic, p=ib - 1: proj_part(p, 1, yparts[p])
                               if (jp == 4 and ic == 0) else None)
                    elif h == 3:
                        mid = (lambda jp, ic, p=ib: proj_part(p, 0, yparts[p])
                               if (jp == 4 and ic == 0) else None)
                    else:
                        mid = None
                    last = (ib == NI // IB - 1 and h == 3)
                    if last:
                        prev_mid = mid
                        def mid(jp, ic, pm=prev_mid, p=ib):
                            if pm is not None:
                                pm(jp, ic)
                            if ic == 1 and jp == 8:
                                proj_part(p, 1, yparts[p], halves=(0,), final=True)
                        post = (lambda ic, p=ib: proj_part(
                            p, 1, yparts[p], halves=(1,), final=True)
                            if ic == 1 else None)
                    else:
                        post = None
                    prev2 = mid
                    def mid(jp, ic, pm=prev2, eps_=tuple(pending_ep)):
                        if ic == 0 and jp == 2:
                            for e in eps_:
                                e()
                        if pm is not None:
                            pm(jp, ic)
                    pending_ep = attn_head(
                        ib, h, with_vt=(ib == 0 and h == 0), mid_cb=mid,
                        post_ic=post, defer_last=not last)
                    if ib == 0 and h == 1:
                        exp_early[0] = False

    nc.compile()
    return nc


def _prep_in_maps(x, gn_w, gn_b, qkv_w, qkv_b, proj_w, proj_b):
    x = np.ascontiguousarray(np.asarray(x, np.float32)).reshape(B, C, N)
    qkv_w = np.asarray(qkv_w, np.float32)
    qkv_b = np.asarray(qkv_b, np.float32)
    proj_w = np.asarray(proj_w, np.float32)
    proj_b = np.asarray(proj_b, np.float32)
    gn_w = np.asarray(gn_w, np.float32)
    gn_b = np.asarray(gn_b, np.float32)

    bf = ml_dtypes.bfloat16
    qs = SCHR_A / 8.0             # fold D^-0.5 and the Schraudolph slope into q
    wq = qkv_w[:C] * qs
    wk = qkv_w[C : 2 * C]
    wv = 16.0 * qkv_w[2 * C :]    # scale v for fp8; /16 folded into wpT
    wqkvT = np.ascontiguousarray(np.concatenate([wq.T, wk.T, wv.T], axis=1)).astype(bf)
    wpT = np.ascontiguousarray(proj_w.T / 16.0).astype(bf)
    qkb = np.ascontiguousarray(
        np.concatenate([(qkv_b[:C] * qs).reshape(2, 128).T,
                        qkv_b[C : 2 * C].reshape(2, 128).T], axis=1))
    # fold v-bias through proj: proj(att + vb) = proj(att) + proj_w @ vb
    pb_eff = proj_b + proj_w.astype(np.float64) @ qkv_b[2 * C :].astype(np.float64)
    pb = np.ascontiguousarray(pb_eff.astype(np.float32).reshape(2, 128).T)
    gnw2 = np.ascontiguousarray(gn_w.reshape(2, 128).T)
    gnb2 = np.ascontiguousarray(gn_b.reshape(2, 128).T)
    cidx = np.arange(C)
    oneh = (cidx[:, None] // 32 == np.arange(GROUPS)[None, :]).astype(np.float32) / 32.0
    onehT = np.ascontiguousarray(oneh.T * 32.0)
    zz = np.zeros((128, N), ml_dtypes.float8_e4m3)

    shared = {
        "wqkvT": wqkvT, "wpT": wpT, "qkb": qkb, "pb": pb,
        "gnw": gnw2, "gnb": gnb2, "oneh": oneh, "onehT": onehT, "zz": zz,
    }
    in_maps = []
    for core in range(NCORES):
        bi, half = divmod(core, 2)
        xb = x[bi]
        if half:
            xs = np.concatenate([xb[:, NI:], xb[:, :NI]], axis=1)
        else:
            xs = xb
        in_maps.append({"x": np.ascontiguousarray(xs.astype(bf)), **shared})
    return in_maps


def _assemble(results):
    y = np.empty((B, C, N), np.float32)
    for core in range(NCORES):
        bi, half = divmod(core, 2)
        y[bi][:, half * NI : (half + 1) * NI] = results[core]["out"]
    return y.reshape(B, C, H, W)


def kernel(x, gn_w, gn_b, qkv_w, qkv_b, proj_w, proj_b):
    from concourse.bass_utils import run_bass_kernel_spmd

    if "nc" not in _CACHE:
        _CACHE["nc"] = _build_nc()
    nc = _CACHE["nc"]
    in_maps = _prep_in_maps(x, gn_w, gn_b, qkv_w, qkv_b, proj_w, proj_b)
    res = run_bass_kernel_spmd(nc, in_maps, core_ids=list(range(NCORES)))
    return _assemble(res.results)


# revision 26
# speedup vs baseline: 1.0256x; 1.0148x over previous
"""AttentionBlock (GroupNorm -> 1x1 qkv -> 4-head attention over 64x64 -> proj -> residual)
distributed over 8 Trainium2 NeuronCores.

Sharding: 8 shards = batch(4) x query-half(2). Each core receives the full
[256, 4096] feature map of its batch element (columns rolled so its own query
half is always columns 0:2048 -> identical SPMD graph on every core).

Key optimizations over the bf16 baseline (366 us):
- GroupNorm folded into runtime-scaled qkv weights (W' = a_c * W, bias via tiny
  PE matvecs); x cast to bf16 once, no normalized-x materialization.
- The whole attention inner loop runs in fp8e4 DoubleRow matmuls (2x contraction
  per column): scores contract q against both k head-planes with a zero-plane
  masking trick (q8 planes [q0,q1,0,q2,q3]; head h reads plane pair (h-plane,
  zero-plane) so the unwanted head contributes nothing), and attn@v contracts
  two 128-key j-chunks per matmul. K>=65 tiles keep the PE in full-rate 128-row
  mode (K<=64 streams at half rate on TRN2).
- exp(scores) split between Scalar (exact Exp -> fp8, table-free) and Vector
  (Schraudolph: scores arrive pre-scaled by 11.5416 = 8/ln2 via the host q
  scale, so max(psum + c0, 0) cast to uint8 IS the fp8 bit pattern of
  e^(s - DELTA)). GPSIMD cannot read PSUM, so it handles SBUF-only work.
- softmax denominator via a ones column in the fp8 v^T (row 64 of the DoubleRow
  pv matmul), normalization off the critical path via a DRAM-broadcast round
  trip, v-bias and all GroupNorm bias terms pushed into the proj bias.
"""

import sys

sys.path.insert(0, "/opt/trn_rl_repo")

import numpy as np
import ml_dtypes

import concourse.bass as bass
import concourse.tile as tile
from concourse import bacc, mybir

# Problem geometry (hardcoded per harness contract)
B, C, H, W = 4, 256, 64, 64
N = H * W              # 4096 spatial positions
HEADS = 4
D = C // HEADS         # 64
GROUPS = 8
EPS = 1e-5
NCORES = 8
NI = N // 2            # 2048 queries per core
IB = 1024              # i-block
JC = 128               # j-chunk (keys per scores matmul)
NJ = N // JC           # 32 j-chunks
NJP = NJ // 2          # 16 j-chunk pairs (DoubleRow contracts a pair)

DELTA = 2.0                       # score shift: exp(s - DELTA), cancels in softmax
SCHR_A = 8.0 / float(np.log(2))   # 11.5416 = fp8e4 bits per e-fold
SCHR_C0 = 56.3                    # bits = A*(s - DELTA) + C0

F32 = mybir.dt.float32
BF16 = mybir.dt.bfloat16
F8 = mybir.dt.float8e4
U8 = mybir.dt.uint8
DR = mybir.MatmulPerfMode.DoubleRow

# q8 planes: [q0, q1, zero, q2, q3]; head h -> (first plane, plane stride)
Q_PLANES = [(0, 2), (1, 1), (2, 1), (2, 2)]

_CACHE = {}


def _build_nc():
    nc = bacc.Bacc("TRN2", target_bir_lowering=False, debug=False,
                   num_devices=NCORES)

    x_ext = nc.declare_dram_parameter("x", [C, N], BF16, isOutput=False)
    # columns: [ (A/8)*qT (256) | kT (256) | 16*vT (256) ],  A = 8/ln2
    wqkvT_ext = nc.declare_dram_parameter("wqkvT", [C, 3 * C], BF16, isOutput=False)
    wpT_ext = nc.declare_dram_parameter("wpT", [C, C], BF16, isOutput=False)
    qkb_ext = nc.declare_dram_parameter("qkb", [128, 4], F32, isOutput=False)
    pb_ext = nc.declare_dram_parameter("pb", [128, 2], F32, isOutput=False)
    gnw_ext = nc.declare_dram_parameter("gnw", [128, 2], F32, isOutput=False)
    gnb_ext = nc.declare_dram_parameter("gnb", [128, 2], F32, isOutput=False)
    oneh_ext = nc.declare_dram_parameter("oneh", [C, GROUPS], F32, isOutput=False)
    onehT_ext = nc.declare_dram_parameter("onehT", [GROUPS, C], F32, isOutput=False)
    zz_ext = nc.declare_dram_parameter("zz", [128, N], F8, isOutput=False)
    out_ext = nc.declare_dram_parameter("out", [C, NI], F32, isOutput=True)

    with tile.TileContext(nc) as tc:
        with (
            tc.tile_pool(name="persist", bufs=1) as per,
            tc.tile_pool(name="etp", bufs=6) as etp,
            tc.tile_pool(name="ep", bufs=2) as ep,
            tc.tile_pool(name="yp", bufs=2) as yp,
            tc.tile_pool(name="dp", bufs=2, space="DRAM") as dp,
            tc.tile_pool(name="ps", bufs=3, space="PSUM") as ps,
            tc.tile_pool(name="pv", bufs=2, space="PSUM") as pvp,
        ):
            # ---- persistent SBUF tensors ----
            x_sb = [per.tile([128, N], BF16, tag=f"x{t}", name=f"x{t}") for t in range(2)]
            q8 = per.tile([128, 5, NI], F8, tag="q8")
            k8 = per.tile([128, 2, N], F8, tag="k8")
            # v^T per j-chunk: [j%128, jchunk, head, 0:64 v | 64 ones | 65 zero]
            vton = per.tile([128, NJ, HEADS, 128], F8, tag="vton")
            att_sb = [per.tile([128, NI], BF16, tag=f"att{t}", name=f"att{t}") for t in range(2)]
            wraw_sb = [per.tile([128, 3 * C], BF16, tag=f"wr{t}", name=f"wr{t}") for t in range(2)]
            wsc_sb = [per.tile([128, 3 * C], BF16, tag=f"ws{t}", name=f"ws{t}") for t in range(2)]
            wpT_sb = [per.tile([128, C], BF16, tag=f"wp{t}", name=f"wp{t}") for t in range(2)]
            qkb_sb = per.tile([128, 4], F32, tag="qkb")
            biasqk_sb = per.tile([128, 4], F32, tag="biasqk")
            pb_sb = per.tile([128, 2], F32, tag="pb")
            pbrt_sb = per.tile([128, 2], F32, tag="pbrt")
            gnw_sb = per.tile([128, 2], F32, tag="gnw")
            gnb_sb = per.tile([128, 2], F32, tag="gnb")
            oneh_sb = [per.tile([128, GROUPS], F32, tag=f"oneh{t}", name=f"oneh{t}") for t in range(2)]
            onehT_sb = per.tile([GROUPS, C], F32, tag="onehT")
            eps_sb = per.tile([GROUPS, 1], F32, tag="eps")
            ndel_sb = per.tile([128, 1], F32, tag="ndel")
            ab_sb = [per.tile([128, 2], F32, tag=f"ab{t}", name=f"ab{t}") for t in range(2)]
            bvec_sb = [per.tile([128, 1], BF16, tag=f"bv{t}", name=f"bv{t}") for t in range(2)]
            vx16_sb = [per.tile([128, 1], BF16, tag=f"vx{t}", name=f"vx{t}") for t in range(2)]
            gst_sb = per.tile([GROUPS, 4], F32, tag="gst")

            ones1 = per.tile([1, D], F32, tag="ones1")
            nc.vector.memset(ones1[:], 1.0)
            nc.vector.memset(eps_sb[:], EPS)
            nc.vector.memset(ndel_sb[:], -DELTA)
            nc.vector.memset(vton[:, :, :, D : D + 1], 1.0)
            nc.vector.memset(vton[:, :, :, D + 1 : D + 2], 0.0)

            # ---- x DMA + GroupNorm statistics + bf16 cast ----
            stats = [per.tile([128, 8, 6], F32, tag=f"st{t}", name=f"st{t}") for t in range(2)]
            mv = [per.tile([128, 4], F32, tag=f"mv{t}", name=f"mv{t}") for t in range(2)]
            x_queues = {(0, 0): nc.sync, (0, 1): nc.sync,
                        (0, 2): nc.sync, (0, 3): nc.scalar,
                        (1, 0): nc.gpsimd, (1, 1): nc.gpsimd,
                        (1, 2): nc.gpsimd, (1, 3): nc.scalar}
            for t in range(2):
                cs = slice(t * 128, (t + 1) * 128)
                for ch in range(4):
                    chs = slice(ch * IB, (ch + 1) * IB)
                    x_queues[(t, ch)].dma_start(out=x_sb[t][:, chs], in_=x_ext[cs, chs])
                    for s in range(2):
                        sub = ch * 2 + s
                        nc.vector.bn_stats(
                            out=stats[t][:, sub, :],
                            in_=x_sb[t][:, sub * 512 : (sub + 1) * 512],
                        )
                if t == 1:
                    # weights / zeros / small inputs ride behind x
                    for t2 in range(2):
                        cs2 = slice(t2 * 128, (t2 + 1) * 128)
                        nc.sync.dma_start(out=wraw_sb[t2][:], in_=wqkvT_ext[cs2, :])
                        nc.scalar.dma_start(out=wpT_sb[t2][:], in_=wpT_ext[cs2, :])
                        nc.scalar.dma_start(out=oneh_sb[t2][:], in_=oneh_ext[cs2, :])
                    nc.scalar.dma_start(out=qkb_sb[:], in_=qkb_ext[:])
                    nc.scalar.dma_start(out=pb_sb[:], in_=pb_ext[:])
                    nc.sync.dma_start(out=gnw_sb[:], in_=gnw_ext[:])
                    nc.sync.dma_start(out=gnb_sb[:], in_=gnb_ext[:])
                    nc.scalar.dma_start(out=onehT_sb[:], in_=onehT_ext[:])
                    nc.gpsimd.dma_start(out=k8[:, 1, :], in_=zz_ext[:])
                    nc.gpsimd.dma_start(out=q8[:, 2, :], in_=zz_ext[:, 0:NI])
                    nc.gpsimd.dma_start(out=q8[64:128, 0, :], in_=zz_ext[64:128, 0:NI])
                    nc.gpsimd.dma_start(out=q8[0:64, 1, :], in_=zz_ext[0:64, 0:NI])
                    nc.gpsimd.dma_start(out=q8[64:128, 3, :], in_=zz_ext[64:128, 0:NI])
                    nc.gpsimd.dma_start(out=q8[0:64, 4, :], in_=zz_ext[0:64, 0:NI])
                nc.vector.bn_aggr(out=mv[t][:, 0:2], in_=stats[t][:])
                nc.vector.tensor_copy(mv[t][:, 2:3], mv[t][:, 0:1])
                nc.vector.tensor_mul(mv[t][:, 3:4], mv[t][:, 0:1], mv[t][:, 0:1])
                nc.vector.tensor_add(mv[t][:, 3:4], mv[t][:, 1:2], mv[t][:, 3:4])

            # PE warm-up: keep the tensor engine streaming (and its
            # p-state ramped) while x finishes arriving and the GroupNorm
            # chain runs; results are discarded
            for r in range(20):
                wup = ps.tile([128, 2, 512], F32, tag="ps", name=f"wup{r}")
                nc.tensor.matmul(wup[:, 0, :], wraw_sb[0][:, 0:128],
                                 wraw_sb[0][:, 0:512], start=True, stop=True)

            # group means of (mean, E[x^2]): [8, 2]
            gpt = ps.tile([128, 2, 512], F32, tag="ps", name="gn_ps")
            gp = gpt[0:GROUPS, 0, 0:2]
            for t in range(2):
                nc.tensor.matmul(
                    gp, oneh_sb[t][:], mv[t][:, 2:4],
                    start=(t == 0), stop=(t == 1),
                )
            # gst columns: 0=mean_g, 1=rstd_g; scratch 2=var, 3=std
            nc.vector.tensor_copy(gst_sb[:, 0:1], gp[:, 0:1])
            nc.vector.tensor_mul(gst_sb[:, 2:3], gst_sb[:, 0:1], gst_sb[:, 0:1])
            nc.vector.tensor_sub(gst_sb[:, 2:3], gp[:, 1:2], gst_sb[:, 2:3])
            # rstd = exp(-0.5*ln(var+eps)); Ln/Exp share one ACT table set
            nc.scalar.activation(
                out=gst_sb[:, 3:4], in_=gst_sb[:, 2:3],
                func=mybir.ActivationFunctionType.Ln,
                bias=eps_sb[:], scale=1.0,
            )
            nc.vector.tensor_scalar_mul(
                out=gst_sb[:, 3:4], in0=gst_sb[:, 3:4], scalar1=-0.5
            )
            nc.scalar.activation(
                out=gst_sb[:, 1:2], in_=gst_sb[:, 3:4],
                func=mybir.ActivationFunctionType.Exp, scale=1.0,
            )

            # broadcast (mean_g, rstd_g) to channels; a = rstd*gn_w,
            # b = gn_b - mean*a; scale weights: ws = a_c * wraw (on GPSIMD)
            bct = ps.tile([128, 2, 512], F32, tag="ps", name="gn_bc")
            for t in range(2):
                bc = bct[:, t, 0:2]
                nc.tensor.matmul(
                    bc, onehT_sb[:, t * 128 : (t + 1) * 128], gst_sb[:, 0:2],
                    start=True, stop=True,
                )
                nc.vector.tensor_mul(ab_sb[t][:, 0:1], bc[:, 1:2], gnw_sb[:, t : t + 1])
                nc.vector.tensor_mul(ab_sb[t][:, 1:2], bc[:, 0:1], ab_sb[t][:, 0:1])
                nc.vector.tensor_sub(ab_sb[t][:, 1:2], gnb_sb[:, t : t + 1], ab_sb[t][:, 1:2])
                nc.vector.tensor_copy(bvec_sb[t][:], ab_sb[t][:, 1:2])
                nc.vector.tensor_scalar_mul(
                    out=wsc_sb[t][:], in0=wraw_sb[t][:], scalar1=ab_sb[t][:, 0:1]
                )

            # runtime bias matvecs: bp[:, 0:6] = wraw.T @ b_vec (per o-tile)
            bpt = ps.tile([128, 2, 512], F32, tag="ps", name="bias_mv")
            bp = bpt[:, 0, 0:6]
            for ot in range(6):
                for t in range(2):
                    nc.tensor.matmul(
                        bp[:, ot : ot + 1],
                        wraw_sb[t][:, ot * 128 : (ot + 1) * 128],
                        bvec_sb[t][:],
                        start=(t == 0), stop=(t == 1),
                    )
            nc.vector.tensor_add(biasqk_sb[:], qkb_sb[:], bp[:, 0:4])
            for t in range(2):
                nc.vector.tensor_copy(vx16_sb[t][:], bp[:, 4 + t : 5 + t])
            # pb_rt = pb + (proj_w/16) @ (16 Wv b_vec)
            pp2t = ps.tile([128, 2, 512], F32, tag="ps", name="pb_mv")
            pp2 = pp2t[:, 0, 0:2]
            for ot in range(2):
                for t in range(2):
                    nc.tensor.matmul(
                        pp2[:, ot : ot + 1],
                        wpT_sb[t][:, ot * 128 : (ot + 1) * 128],
                        vx16_sb[t][:],
                        start=(t == 0), stop=(t == 1),
                    )
            nc.vector.tensor_add(pbrt_sb[:], pb_sb[:], pp2[:, 0:2])

            # ---- emission helpers ----
            def q_rhs(h, isl):
                p0, st = Q_PLANES[h]
                base = q8[:, 0, isl]
                return bass.AP(
                    tensor=base.tensor,
                    offset=base.offset + p0 * NI,
                    ap=[list(base.ap[0])] + [[st * NI, 2]] +
                       [list(a) for a in base.ap[1:]],
                )

            def qkv_tiles(ot, blocks=None):
                # ot 0,1 = q o-tiles (head pairs); 2,3 = k o-tiles
                ncols = NI if ot < 2 else N
                wcols = slice(ot * 128, (ot + 1) * 128)
                for nb in (range(ncols // IB) if blocks is None else blocks):
                    pp = ps.tile([128, 2, 512], F32, tag="ps", name=f"qkv{ot}_{nb}")
                    for cc in range(2):
                        for nh in range(2):
                            nsl = slice(nb * IB + nh * 512, nb * IB + (nh + 1) * 512)
                            nc.tensor.matmul(
                                pp[:, nh, :], wsc_sb[cc][:, wcols], x_sb[cc][:, nsl],
                                start=(cc == 0), stop=(cc == 1),
                            )
                    nbsl = slice(nb * IB, (nb + 1) * IB)
                    ppf = pp[:].rearrange("p a b -> p (a b)")
                    if ot >= 2:
                        nc.scalar.activation(
                            out=k8[:, ot - 2, nbsl], in_=ppf,
                            func=mybir.ActivationFunctionType.Identity,
                            scale=1.0, bias=biasqk_sb[:, ot : ot + 1],
                        )
                    else:
                        # heads 2*ot (rows 0:64) and 2*ot+1 (rows 64:128) go to
                        # their own q8 planes
                        pl0 = [0, 3][ot]
                        pl1 = [1, 4][ot]
                        nc.vector.tensor_scalar_add(
                            out=q8[0:64, pl0, nbsl], in0=ppf[0:64],
                            scalar1=biasqk_sb[0:64, ot : ot + 1],
                        )
                        nc.vector.tensor_scalar_add(
                            out=q8[64:128, pl1, nbsl], in0=ppf[64:128],
                            scalar1=biasqk_sb[64:128, ot : ot + 1],
                        )

            def vt_pair(jp2):
                # v^T for j-chunks (2*jp2, 2*jp2+1): two [j128, 256] matmul
                # groups -> one fp8 copy
                pj = ps.tile([128, 2, 512], F32, tag="ps", name=f"vt{jp2}")
                for jc in range(2):
                    jsl = slice((2 * jp2 + jc) * JC, (2 * jp2 + jc + 1) * JC)
                    for cc in range(2):
                        nc.tensor.matmul(
                            pj[:, jc, 0:256], x_sb[cc][:, jsl],
                            wsc_sb[cc][:, 512:768],
                            start=(cc == 0), stop=(cc == 1),
                        )
                nc.scalar.activation(
                    out=vton[:, 2 * jp2 : 2 * jp2 + 2, :, 0:D],
                    in_=pj[:, :, 0:256].rearrange("p a (h d) -> p a h d", h=HEADS),
                    func=mybir.ActivationFunctionType.Copy, scale=1.0,
                )

            # exp rotation: ACT exact exp -> fp8; DVE Schraudolph (psum is
            # A*s'; the fp8 bits of e^(s'-DELTA) are max(psum+c0, 0) cast to
            # uint8 -- saturating on hw, max() keeps the sim's wrapping cast
            # safe too). 18:14 ratio balances measured engine loads.
            exp_rota = ([True, False] * 14) + [True] * 4
            exp_rota_early = [True, False]
            exp_ctr = [0]
            exp_early = [True]

            def emit_exp(et, scp):
                rota = exp_rota_early if exp_early[0] else exp_rota
                on_act = rota[exp_ctr[0] % len(rota)]
                exp_ctr[0] += 1
                if on_act:
                    nc.scalar.activation(
                        out=et[:], in_=scp[:],
                        func=mybir.ActivationFunctionType.Exp,
                        scale=1.0 / SCHR_A, bias=ndel_sb[:],
                    )
                else:
                    nc.vector.tensor_scalar(
                        out=et[:].bitcast(U8), in0=scp[:],
                        scalar1=SCHR_C0 - SCHR_A * DELTA, scalar2=0.0,
                        op0=mybir.AluOpType.add, op1=mybir.AluOpType.max,
                    )

            def attn_head(ib, h, with_vt, mid_cb=None, post_ic=None,
                          defer_last=False):
                ht = h // 2
                prow = slice((h % 2) * D, (h % 2) * D + D)
                deferred = []
                for ic in range(2):
                    isl = slice(ib * IB + ic * 512, ib * IB + (ic + 1) * 512)
                    pv = pvp.tile([D + 2, 512], F32, tag="pv",
                                  name=f"pv{ib}_{h}_{ic}")
                    pend = None
                    for jp in range(NJP):
                        scp = ps.tile([128, 2, 512], F32, tag="ps",
                                      name=f"sc{ib}_{h}_{jp}_{ic}")
                        et = etp.tile([128, 2, 512], F8, tag="et",
                                      name=f"et{ib}_{h}_{jp}_{ic}")
                        for jc in range(2):
                            j = 2 * jp + jc
                            jsl = slice(j * JC, (j + 1) * JC)
                            nc.tensor.matmul(
                                scp[:, jc, :], k8[:, :, jsl], q_rhs(h, isl),
                                start=True, stop=True, perf_mode=DR,
                            )
                        emit_exp(et, scp)
                        if with_vt and ic == 0 and 3 <= jp <= 14:
                            vt_pair(jp + 1)
                        if mid_cb is not None:
                            mid_cb(jp, ic)
                        # pv for the PREVIOUS jp: one unit of emission lag so
                        # its exp has finished by the time the PE reaches it
                        if pend is not None:
                            pjp, pet = pend
                            nc.tensor.matmul(
                                pv[:], vton[:, 2 * pjp : 2 * pjp + 2, h, 0 : D + 2],
                                pet[:], start=(pjp == 0), stop=False,
                                perf_mode=DR,
                            )
                        pend = (jp, et)
                    pjp, pet = pend
                    nc.tensor.matmul(
                        pv[:], vton[:, 2 * pjp : 2 * pjp + 2, h, 0 : D + 2],
                        pet[:], start=False, stop=True, perf_mode=DR,
                    )
                    def epilogue(ic=ic, pv=pv):
                        _epilogue(ib, h, ht, prow, ic, pv, post_ic)
                    if defer_last and ic == 1:
                        deferred.append(epilogue)
                    else:
                        epilogue()
                return deferred

            def _epilogue(ib, h, ht, prow, ic, pv, post_ic):
                    isl = slice(ib * IB + ic * 512, ib * IB + (ic + 1) * 512)
                    # per-half epilogue: normalize off the critical path.
                    # h<3: den broadcast across 64 partitions via a DRAM round
                    # trip. h==3 (proj waits on it): recip the den row in SBUF
                    # and broadcast through the PE with a ones column instead.
                    if h == 3:
                        denr = ep.tile([1, 512], F32, tag="denr",
                                       name=f"denr{ib}_{h}_{ic}")
                        nc.scalar.activation(
                            out=denr[:], in_=pv[D : D + 1, :],
                            func=mybir.ActivationFunctionType.Copy, scale=1.0)
                        nc.vector.reciprocal_approx_fast(out=denr[:], in_=denr[:])
                        rbp = ps.tile([128, 2, 512], F32, tag="ps",
                                      name=f"rbp{ib}_{h}_{ic}")
                        nc.tensor.matmul(rbp[0:D, 0, :], ones1[:], denr[:],
                                         start=True, stop=True)
                        pvs3 = ep.tile([D, 512], F32, tag="pvs3",
                                       name=f"pvs3{ib}_{h}_{ic}")
                        nc.vector.tensor_copy(pvs3[:], pv[0:D, :])
                        nc.vector.tensor_mul(att_sb[ht][prow, isl],
                                             pvs3[:], rbp[0:D, 0, :])
                    else:
                        pvs = ep.tile([D + 2, 512], F32, tag="pvs",
                                      name=f"pvs{ib}_{h}_{ic}")
                        if ic == 0:
                            nc.scalar.activation(
                                out=pvs[:], in_=pv[:],
                                func=mybir.ActivationFunctionType.Copy, scale=1.0)
                        else:
                            nc.vector.tensor_copy(pvs[:], pv[:])
                        dent = dp.tile([1, 512], F32, tag="dent",
                                       name=f"den{ib}_{h}_{ic}")
                        nc.gpsimd.dma_start(out=dent[:], in_=pvs[D : D + 1, :])
                        rbs = ep.tile([D, 512], F32, tag="rbs",
                                      name=f"rbs{ib}_{h}_{ic}")
                        dbc = bass.AP(
                            tensor=dent.tensor, offset=dent.offset,
                            ap=[[0, D]] + [list(a) for a in dent.ap[1:]],
                        )
                        nc.gpsimd.dma_start(out=rbs[:], in_=dbc)
                        nc.vector.reciprocal_approx_fast(out=rbs[:], in_=rbs[:])
                        nc.gpsimd.tensor_mul(att_sb[ht][prow, isl],
                                             pvs[0:D, :], rbs[:])
                    if post_ic is not None:
                        post_ic(ic)

            def proj_part(ib, cc, ypart_tiles, halves=(0, 1), final=False):
                for ot in range(2):
                    pp = ps.tile([128, 2, 512], F32, tag="ps",
                                 name=f"pj{ib}_{cc}_{ot}_{halves[0]}")
                    wcols = slice(ot * 128, (ot + 1) * 128)
                    for nh in halves:
                        asl = slice(ib * IB + nh * 512, ib * IB + (nh + 1) * 512)
                        nc.tensor.matmul(
                            pp[:, nh, :], wpT_sb[cc][:, wcols], att_sb[cc][:, asl],
                            start=True, stop=True,
                        )
                    for nh in halves:
                        psl = pp[:, nh, :]
                        ysl = slice(ib * IB + nh * 512, ib * IB + (nh + 1) * 512)
                        csl = slice(nh * 512, (nh + 1) * 512)
                        if cc == 0:
                            if nh == halves[0]:
                                yt = yp.tile([128, IB], F32, tag=f"ypart{ot}",
                                             name=f"ypart{ib}_{ot}")
                                ypart_tiles.append(yt)
                            yt = ypart_tiles[ot]
                            nc.scalar.activation(
                                out=yt[:, csl], in_=psl,
                                func=mybir.ActivationFunctionType.Identity,
                                scale=1.0, bias=pbrt_sb[:, ot : ot + 1])
                        else:
                            y_sb = yp.tile([128, 512], F32, tag="y",
                                           name=f"y{ib}_{ot}_{nh}")
                            add1 = nc.vector if final else nc.gpsimd
                            add1.tensor_add(y_sb[:], ypart_tiles[ot][:, csl],
                                            x_sb[ot][:, ysl])
                            nc.vector.tensor_add(y_sb[:], y_sb[:], psl)
                            nc.sync.dma_start(
                                out=out_ext[ot * 128 : (ot + 1) * 128, ysl],
                                in_=y_sb[:])

            # ---- schedule ----
            qkv_tiles(0)        # q heads 0,1 (q8 planes 0,1)
            for jp2 in range(4):
                vt_pair(jp2)    # v chunks for jp 0..3
            qkv_tiles(2, [0])   # k heads 0,1, first block
            yparts = {}
            pending_ep = []
            for ib in range(NI // IB):
                yparts[ib] = []
                for h in range(HEADS):
                    if ib == 0 and h == 0:
                        # k plane0 blocks 1-3 stream in ahead of first use
                        mid = (lambda jp, ic: qkv_tiles(2, [1 + jp // 4])
                               if (ic == 0 and jp in (0, 4, 8)) else None)
                    elif ib == 0 and h == 1:
                        # q planes 3,4 and k plane1 before heads 2,3
                        def mid(jp, ic):
                            if ic == 0 and jp in (0, 4, 8):
                                qkv_tiles(3, [1 + jp // 4] if jp else [0, 1])
                            elif ic == 1 and jp in (0, 8):
                                qkv_tiles(1, [jp // 8])
                    elif ib > 0 and h == 0:
                        mid = (lambda jp, ic, p=ib - 1: proj_part(p, 1, yparts[p])
                               if (jp == 4 and ic == 0) else None)
                    elif h == 3:
                        mid = (lambda jp, ic, p=ib: proj_part(p, 0, yparts[p])
                               if (jp == 4 and ic == 0) else None)
                    else:
                        mid = None
                    last = (ib == NI // IB - 1 and h == 3)
                    if last:
                        prev_mid = mid
                        def mid(jp, ic, pm=prev_mid, p=ib):
                            if pm is not None:
                                pm(jp, ic)
                            if ic == 1 and jp == 8:
                                proj_part(p, 1, yparts[p], halves=(0,), final=True)
                        post = (lambda ic, p=ib: proj_part(
                            p, 1, yparts[p], halves=(1,), final=True)
                            if ic == 1 else None)
                    else:
                        post = None
                    prev2 = mid
                    def mid(jp, ic, pm=prev2, eps_=tuple(pending_ep)):
                        if ic == 0 and jp == 2:
                            for e in eps_:
                                e()
                        if pm is not None:
                            pm(jp, ic)
                    pending_ep = attn_head(
                        ib, h, with_vt=(ib == 0 and h == 0), mid_cb=mid,
                        post_ic=post, defer_last=not last)
                    if ib == 0 and h == 1:
                        exp_early[0] = False

    nc.compile()
    return nc


def _prep_in_maps(x, gn_w, gn_b, qkv_w, qkv_b, proj_w, proj_b):
    x = np.ascontiguousarray(np.asarray(x, np.float32)).reshape(B, C, N)
    qkv_w = np.asarray(qkv_w, np.float32)
    qkv_b = np.asarray(qkv_b, np.float32)
    proj_w = np.asarray(proj_w, np.float32)
    proj_b = np.asarray(proj_b, np.float32)
    gn_w = np.asarray(gn_w, np.float32)
    gn_b = np.asarray(gn_b, np.float32)

    bf = ml_dtypes.bfloat16
    qs = SCHR_A / 8.0             # fold D^-0.5 and the Schraudolph slope into q
    wq = qkv_w[:C] * qs
    wk = qkv_w[C : 2 * C]
    wv = 16.0 * qkv_w[2 * C :]    # scale v for fp8; /16 folded into wpT
    wqkvT = np.ascontiguousarray(np.concatenate([wq.T, wk.T, wv.T], axis=1)).astype(bf)
    wpT = np.ascontiguousarray(proj_w.T / 16.0).astype(bf)
    qkb = np.ascontiguousarray(
        np.concatenate([(qkv_b[:C] * qs).reshape(2, 128).T,
                        qkv_b[C : 2 * C].reshape(2, 128).T], axis=1))
    # fold v-bias through proj: proj(att + vb) = proj(att) + proj_w @ vb
    pb_eff = proj_b + proj_w.astype(np.float64) @ qkv_b[2 * C :].astype(np.float64)
    pb = np.ascontiguousarray(pb_eff.astype(np.float32).reshape(2, 128).T)
    gnw2 = np.ascontiguousarray(gn_w.reshape(2, 128).T)
    gnb2 = np.ascontiguousarray(gn_b.reshape(2, 128).T)
    cidx = np.arange(C)
    oneh = (cidx[:, None] // 32 == np.arange(GROUPS)[None, :]).astype(np.float32) / 32.0
    onehT = np.ascontiguousarray(oneh.T * 32.0)
    zz = np.zeros((128, N), ml_dtypes.float8_e4m3)

    shared = {
        "wqkvT": wqkvT, "wpT": wpT, "qkb": qkb, "pb": pb,
        "gnw": gnw2, "gnb": gnb2, "oneh": oneh, "onehT": onehT, "zz": zz,
    }
    in_maps = []
    for core in range(NCORES):
        bi, half = divmod(core, 2)
        xb = x[bi]
        if half:
            xs = np.concatenate([xb[:, NI:], xb[:, :NI]], axis=1)
        else:
            xs = xb
        in_maps.append({"x": np.ascontiguousarray(xs.astype(bf)), **shared})
    return in_maps


def _assemble(results):
    y = np.empty((B, C, N), np.float32)
    for core in range(NCORES):
        bi, half = divmod(core, 2)
        y[bi][:, half * NI : (half + 1) * NI] = results[core]["out"]
    return y.reshape(B, C, H, W)


def kernel(x, gn_w, gn_b, qkv_w, qkv_b, proj_w, proj_b):
    from concourse.bass_utils import run_bass_kernel_spmd

    if "nc" not in _CACHE:
        _CACHE["nc"] = _build_nc()
    nc = _CACHE["nc"]
    in_maps = _prep_in_maps(x, gn_w, gn_b, qkv_w, qkv_b, proj_w, proj_b)
    res = run_bass_kernel_spmd(nc, in_maps, core_ids=list(range(NCORES)))
    return _assemble(res.results)
